# revision 5
# baseline (speedup 1.0000x reference)
"""Trainium2 Bass kernel for nn_PamCell (spatial self-attention, B=4, C=64,
N=16^3=4096, CQ=8) on 8 NeuronCores.

Sharding: core i handles batch i//2 and query-half i%2 (2048 queries vs all
4096 keys). No collectives; host scatters inputs / gathers outputs.

Math: softmax rows are invariant to additive terms that depend only on the
query index, so with A = wq^T wk and u = wk^T bq,
    softmax(q k^T)[n, :] == softmax((A^T x_n + u) . x_m)[n, :]
which turns the QK contraction into a single 64-dim contraction against the
raw input as keys. Energies are in [-5, 5]: the softmax max-subtraction is
skipped (exp cannot overflow).

v2 changes vs the 119.5us baseline (which was ACT-serialized: the scalar
engine ran every exp tile, ~2.2us/granule, while the PE idled):
  - exp is split across engines per granule: ACT does the even key chunk
    (hardware Exp), DVE does the odd chunk with a magic-constant exp
    (bf16 bits of 2^t are linear in t: int16(e*184.665 + 16250.49) bitcast
    to bf16, +-3% sawtooth; RNE convert verified on HW).
  - ~4us of dummy matmuls at the head of the PE queue warm the HAM clock
    gate (2.4GHz) before the first real matmul instead of 15us in.
  - input DMAs spread over 4 engine queues; xq_f32 (epilogue-only) last.
  - bv is pre-loaded into the v^T PSUM accumulation by a K=1 ones matmul,
    dropping the DVE broadcast-add from the prologue.
  - granules run query-half-major so half 0's 1/rowsum (Ln+Exp on ACT, same
    table set) hides under half 1's main loop; the remaining tail is the
    4x [K=1 broadcast matmul -> ACT copy -> DVE mult/add -> DMA] pipeline.
"""

import sys

import numpy as np

try:
    import concourse.bass as bass
except ImportError:  # fresh interpreter without the env paths
    for _p in ("/root/.axon_site", "/root/.axon_site/_ro/trn_rl_repo",
               "/root/.axon_site/_ro/pypackages", "/opt/trn_rl_repo"):
        if _p not in sys.path:
            sys.path.append(_p)
    import concourse.bass as bass

import ml_dtypes

import concourse.tile as tile
from concourse import mybir
from concourse.vector_clock import ScopedClock

BF16 = mybir.dt.bfloat16
F32 = mybir.dt.float32
I16 = mybir.dt.int16
AF = mybir.ActivationFunctionType

B, C, N = 4, 64, 4096
NQ = N // 2          # queries per core
NKC = N // 128       # key chunks of 128
N_CORES = 8
NPAIR = NKC // 2     # 16 key-chunk pairs

S_MAGIC = 128.0 / float(np.log(2.0))   # 184.6650
B_MAGIC = 16256.0 - 5.5078             # RNE-rounded magic bias (HW-verified)
N_WARM = 18                            # dummy warm matmuls, 256 cols each


class _TileContextCompat(tile.TileContext):
    """Split the kernel-tail drain's sem waits across SP instructions;
    this walrus build allows only one sync-wait per CTRL instruction."""

    def _drain_and_barrier(self, tick_clock, wait_clock):
        probe = self.nc.sync.nop()
        wait_clock.add_sem_waits(
            probe.ins, ScopedClock({None: tick_clock.global_clock})
        )
        si = probe.ins.sync_info
        waits = list(si.on_wait) if si is not None else []
        if si is not None:
            probe.ins.sync_info = mybir.SyncInfo(
                on_wait=waits[:1], on_update=list(si.on_update)
            )
        for w in waits[1:]:
            nop = self.nc.sync.nop()
            nop.ins.sync_info = mybir.SyncInfo(on_wait=[w], on_update=[])

        self.nc.sync.drain()
        self.nc.all_engine_barrier()
        assert self.sems is not None
        popped = self.nc._tile_sem_poison_stack.pop()
        assert popped is self._sem_poison
        self.nc.clear_and_free_semaphores(list(self.sems.allocated().values()))
        self.nc.all_engine_barrier()


def _split_sync_waits(nc, max_waits=1):
    """This walrus build rejects instructions carrying more than one sync
    wait; hoist excess waits onto same-engine nops inserted just before."""
    for fn in nc.m.functions:
        for blk in fn.blocks:
            new = []
            changed = False
            for inst in blk.instructions:
                si = inst.sync_info
                if si is not None and si.on_wait and len(si.on_wait) > max_waits:
                    waits = list(si.on_wait)
                    excess = waits[:-max_waits]
                    for i in range(0, len(excess), max_waits):
                        nop = mybir.InstNoOp(
                            name=f"I-{nc.next_id()}-waitsplit", ins=[], outs=[]
                        )
                        nop.engine = inst.engine
                        nop.sync_info = mybir.SyncInfo(
                            on_wait=excess[i : i + max_waits], on_update=[]
                        )
                        new.append(nop)
                    inst.sync_info = mybir.SyncInfo(
                        on_wait=waits[-max_waits:], on_update=list(si.on_update)
                    )
                    changed = True
                new.append(inst)
            if changed:
                blk.instructions = new


def build_nc(split=True):
    nc = bass.Bass(
        "TRN2",
        target_bir_lowering=False,
        debug=False,
        enable_asserts=False,
    )
    xk_bf = nc.dram_tensor("xk_bf", (C, N), BF16, kind="ExternalInput")
    xkd_bf = nc.dram_tensor("xkd_bf", (C, N), BF16, kind="ExternalInput")
    xq_bf = nc.dram_tensor("xq_bf", (C, NQ), BF16, kind="ExternalInput")
    xq_f32 = nc.dram_tensor("xq_f32", (C, NQ), F32, kind="ExternalInput")
    a_aug = nc.dram_tensor("a_aug", (C + 1, C), BF16, kind="ExternalInput")
    wv_dup = nc.dram_tensor("wv_dup", (128, C), BF16, kind="ExternalInput")
    bv8 = nc.dram_tensor("bv8", (1, 512), BF16, kind="ExternalInput")
    out = nc.dram_tensor("out", (C, NQ), F32, kind="ExternalOutput")

    with _TileContextCompat(nc) as tc:
        with tc.tile_pool(name="consts", bufs=1) as consts:
            # ---- persistent SBUF tensors ----
            xk2 = consts.tile([128, N], BF16, tag="xk2")     # keys, dup rows
            xq = consts.tile([C + 1, NQ], BF16, tag="xq")    # queries + ones
            xqf = consts.tile([C, NQ], F32, tag="xqf")
            a_sb = consts.tile([C + 1, C], BF16, tag="a_sb")
            wv_sb = consts.tile([128, C], BF16, tag="wv_sb")  # wv^T, dup rows
            bv_sb = consts.tile([1, 512], BF16, tag="bv_sb")  # gamma*bv, tiled 8x
            qb2 = consts.tile([128, NQ], BF16, tag="qb2")    # Q, dup rows
            vt = consts.tile([128, NKC, C + 1], BF16, tag="vt")
            ones_kb = consts.tile([1, 128], BF16, tag="ones_kb")
            warm_w = consts.tile([128, 256], BF16, tag="warm_w")
            r_sb = consts.tile([1, NQ], F32, tag="r_sb")
            rb_bf = consts.tile([1, NQ], BF16, tag="rb_bf")
            bc_sb = consts.tile([C, NQ], F32, tag="bc_sb")
            warm_sb = consts.tile([1, 128], F32, tag="warm_sb")

            import bass_rust as _br

            pe_chain = [None]
            act_chain = [None]
            dve_chain = [None]

            def _chained(r, chain, reason="order"):
                if chain[0] is not None:
                    _br.add_dep_helper(r.ins, chain[0].ins, reason=reason)
                chain[0] = r
                return r

            # ---- memsets (vector: tiny consts; gpsimd: larger fills) ----
            nc.vector.memset(ones_kb[:], 1.0)
            nc.gpsimd.memset(warm_w[:], 1.0)
            nc.gpsimd.memset(xq[C : C + 1, :], 1.0)
            nc.gpsimd.memset(vt[:, :, C : C + 1], 1.0)

            # trigger the ~2.7us table load (natural_log set: Ln + Exp + Copy)
            _chained(nc.scalar.activation(warm_sb[:], ones_kb[:], AF.Ln),
                     act_chain)

            # ---- input DMAs, spread across queues ----
            # sync: xq_bf then key quarters 0,1 (query-half-0 energies first)
            nc.sync.dma_start(xq[:C, :], xq_bf.ap())
            nc.sync.dma_start(xk2[:C, bass.ts(0, N // 4)],
                              xk_bf.ap()[:, bass.ts(0, N // 4)])
            nc.sync.dma_start(xk2[:C, bass.ts(1, N // 4)],
                              xk_bf.ap()[:, bass.ts(1, N // 4)])
            # scalar: small weights then key quarters 2,3
            nc.scalar.dma_start(a_sb[:], a_aug.ap())
            nc.scalar.dma_start(wv_sb[:], wv_dup.ap())
            nc.scalar.dma_start(bv_sb[:], bv8.ap())
            nc.scalar.dma_start(xk2[:C, bass.ts(2, N // 4)],
                                xk_bf.ap()[:, bass.ts(2, N // 4)])
            nc.scalar.dma_start(xk2[:C, bass.ts(3, N // 4)],
                                xk_bf.ap()[:, bass.ts(3, N // 4)])
            # sync also takes dup quarter 0 (only SP/ACT/gpsimd can DMA)
            nc.sync.dma_start(xk2[C:, bass.ts(0, N // 4)],
                              xkd_bf.ap()[:, bass.ts(0, N // 4)])
            # gpsimd: dup quarters 1-3 + epilogue-only fp32 queries
            for gq in range(1, 4):
                nc.gpsimd.dma_start(xk2[C:, bass.ts(gq, N // 4)],
                                    xkd_bf.ap()[:, bass.ts(gq, N // 4)])
            nc.gpsimd.dma_start(xqf[:], xq_f32.ap())

            # ---- prologue ----
            # PSUM: q0(2) q1(2) vp(2) warm(1) = 7 banks; released before the
            # main loop so e0/e1/out_big can take all 8.
            with tc.tile_pool(name="psum_pro", bufs=1, space="PSUM") as pro:
                # dummy matmuls: keep the PE busy from t~0 so the HAM clock
                # gate is warm (2.4GHz) when the first real matmul issues.
                # single row group so consecutive warm MMs serialize: two
                # row-tiled MMs writing the same PSUM region would race.
                warm_ps = pro.tile([128, 256], F32, tag="warm_ps")
                for i in range(N_WARM):
                    _chained(nc.tensor.matmul(
                        warm_ps[:, :],
                        warm_w[0:64, 0:128],
                        warm_w[0:64, :],
                        start=True,
                        stop=True,
                        tile_position=(0, 0),
                        skip_group_check=True,
                    ), pe_chain)

                # Q = a_aug^T xq_aug, written twice (col-tiled) so both
                # partition halves hold a copy for the row-tiled energy MMs
                q_halves = [
                    pro.tile([128, NQ // 2], F32, tag=f"q{h}", name=f"q{h}")
                    for h in range(2)
                ]
                for j in range(NQ // 512):
                    q_ps = q_halves[j // 2]
                    js = bass.ts(j % 2, 512)
                    _chained(nc.tensor.matmul(
                        q_ps[:C, js],
                        a_sb[:],
                        xq[:, bass.ts(j, 512)],
                        start=True,
                        stop=True,
                        tile_position=(0, 0),
                    ), pe_chain)
                    _chained(nc.tensor.matmul(
                        q_ps[C:, js],
                        a_sb[:],
                        xq[:, bass.ts(j, 512)],
                        start=True,
                        stop=True,
                        tile_position=(0, 64),
                    ), pe_chain)
                for h in range(2):
                    _chained(nc.vector.tensor_copy(
                        qb2[:, bass.ts(h, NQ // 2)], q_halves[h][:]
                    ), dve_chain)

                # v^T per key chunk, row-tiled pairs; bv is pre-loaded into
                # the accumulation by a K=1 ones matmul so no bias add is
                # needed afterwards. NOT pe-chained: the scheduler slots
                # these into PE gaps while the first exps run.
                vt_r = vt.rearrange("p (t two) c -> p t two c", two=2)
                for g in range(2):
                    vp = pro.tile([128, 1024], F32, tag="vp", bufs=1, name="vp")
                    for half in range(2):
                        nc.tensor.matmul(
                            vp[:, bass.ts(half, 512)],
                            ones_kb[:, :],
                            bv_sb[:, :],
                            start=True,
                            stop=False,
                            skip_group_check=True,
                        )
                    for t in range(8):
                        pair = 8 * g + t
                        nc.tensor.matmul(
                            vp[:, bass.ts(t, C)],
                            xk2[:C, bass.ts(2 * pair, 128)],
                            wv_sb[:C, :],
                            start=False,
                            stop=True,
                            tile_position=(0, 0),
                            skip_group_check=True,
                        )
                        nc.tensor.matmul(
                            vp[:, bass.ds(512 + t * C, C)],
                            xk2[C:, bass.ts(2 * pair + 1, 128)],
                            wv_sb[C:, :],
                            start=False,
                            stop=True,
                            tile_position=(64, 0),
                            skip_group_check=True,
                        )
                    for half in range(2):
                        _chained(nc.scalar.copy(
                            vt_r[:, bass.ts(g, 8), half, :C],
                            vp[:, bass.ts(half, 512)].rearrange(
                                "p (t c) -> p t c", t=8
                            ),
                        ), act_chain)

            # ---- main loop (query-half-major) ----
            with (
                tc.tile_pool(name="psum_e", bufs=1, space="PSUM") as pe_pool,
                tc.tile_pool(name="psum_out", bufs=1, space="PSUM") as pout,
            ):
                out_big = pout.tile([C + 1, NQ], F32, tag="out_big",
                                    name="out_big")
                out_ps = [
                    out_big[:, bass.ts(qg, 512)] for qg in range(NQ // 512)
                ]
                with (
                    tc.tile_pool(name="pt_pool", bufs=6) as pt_pool,
                ):
                    # granule = (pair, qh), qh-major: granules 0-15 cover
                    # query half 0 over all key pairs, 16-31 half 1. Half 0's
                    # epilogue recip overlaps half 1's compute.
                    NG = NKC  # 32 granules
                    gr = [(pair, qh) for qh in range(2) for pair in range(NPAIR)]

                    def energies(g):
                        """All 4 energy MMs of granule g, interleaved h0/h64
                        so adjacent MMs run concurrently in disjoint row
                        groups."""
                        pair, qh = gr[g]
                        qoff = qh * 1024
                        es = []
                        for half in range(2):
                            es.append(pe_pool.tile(
                                [128, 1024], F32, tag=f"e{half}",
                                name=f"e{half}"
                            ))
                        for j in range(2):
                            for half in range(2):
                                mc = 2 * pair + half
                                lo = C * half
                                _chained(
                                    nc.tensor.matmul(
                                        es[half][:, bass.ts(j, 512)],
                                        xk2[lo : lo + C, bass.ts(mc, 128)],
                                        qb2[lo : lo + C,
                                            bass.ds(qoff + j * 512, 512)],
                                        start=True,
                                        stop=True,
                                        tile_position=(lo, 0),
                                    ),
                                    pe_chain,
                                    "pe-order",
                                )
                        return es

                    def do_exp_act(e):
                        """Even chunk: hardware Exp on the scalar engine."""
                        pt = pt_pool.tile([128, 1024], BF16, tag="pt0",
                                          name="pt0")
                        _chained(
                            nc.scalar.activation(pt[:], e[:], AF.Exp),
                            act_chain,
                            "act-order",
                        )
                        return pt

                    def do_exp_dve(e):
                        """Odd chunk: magic-constant exp on the vector
                        engine: bf16 bits of 2^(e/ln2) are int16(e*s + b)."""
                        pt = pt_pool.tile([128, 1024], BF16, tag="pt1",
                                          name="pt1")
                        _chained(
                            nc.vector.tensor_scalar(
                                pt[:].bitcast(I16), e[:], S_MAGIC, B_MAGIC,
                                mybir.AluOpType.mult, mybir.AluOpType.add,
                            ),
                            dve_chain,
                            "dve-order",
                        )
                        return pt

                    def outs(g, half, pt):
                        pair, qh = gr[g]
                        mc = 2 * pair + half
                        for j in range(2):
                            qg = 2 * qh + j
                            _chained(
                                nc.tensor.matmul(
                                    out_ps[qg][:],
                                    vt[:, mc, :],
                                    pt[:, bass.ts(j, 512)],
                                    start=(pair == 0),
                                    stop=(pair == NPAIR - 1),
                                    skip_group_check=True,
                                ),
                                pe_chain,
                                "pe-order",
                            )

                    def recip(qh):
                        """1/rowsum for one query half: Ln then Exp(-x) on
                        ACT (same natural_log table set as the main exps)."""
                        hs = bass.ts(qh, 1024)
                        _chained(nc.scalar.activation(
                            r_sb[:, hs], out_big[C : C + 1, hs], AF.Ln
                        ), act_chain, "act-order")
                        _chained(nc.scalar.activation(
                            rb_bf[:, hs], r_sb[:, hs], AF.Exp, scale=-1.0
                        ), act_chain, "act-order")

                    es = {0: energies(0)}
                    for g in range(NG):
                        eA, eB = es.pop(g)
                        ptA = do_exp_act(eA)
                        ptB = do_exp_dve(eB)
                        if g + 1 < NG:
                            es[g + 1] = energies(g + 1)
                        outs(g, 0, ptA)
                        outs(g, 1, ptB)
                        if g == NPAIR - 1:
                            recip(0)  # half-0 rowsum: hides under half 1
                    recip(1)

                # ---- epilogue ----
                # gamma is folded into v on the host, so the final result is
                # out[c,q] * r[q] + x[c,q] with r = 1/rowsum broadcast to 64
                # partitions by a K=1 ones matmul into the freed energy
                # banks.
                with tc.tile_pool(name="epi", bufs=2) as epi:
                    for qg in range(NQ // 512):
                        bc_ps = pe_pool.tile(
                            [C, 512], F32, tag=f"e{qg % 2}", name=f"bc{qg}"
                        )
                        _chained(nc.tensor.matmul(
                            bc_ps[:],
                            ones_kb[:, :C],
                            rb_bf[:, bass.ts(qg, 512)],
                            start=True,
                            stop=True,
                        ), pe_chain, "pe-order")
                        _chained(nc.scalar.copy(
                            bc_sb[:, bass.ts(qg, 512)], bc_ps[:]
                        ), act_chain, "act-order")
                        t_sb = epi.tile([C, 512], F32, tag="t_sb")
                        _chained(nc.vector.tensor_tensor(
                            t_sb[:], out_ps[qg][:C, :],
                            bc_sb[:, bass.ts(qg, 512)],
                            mybir.AluOpType.mult,
                        ), dve_chain, "dve-order")
                        _chained(nc.vector.tensor_tensor(
                            t_sb[:], t_sb[:], xqf[:, bass.ts(qg, 512)],
                            mybir.AluOpType.add,
                        ), dve_chain, "dve-order")
                        nc.sync.dma_start(out.ap()[:, bass.ts(qg, 512)],
                                          t_sb[:])

    if split:
        _split_sync_waits(nc)
    return nc


def host_prep(inputs):
    """Full inputs -> list of 8 per-core input maps."""
    x = np.asarray(inputs["x"], np.float32)
    wq = np.asarray(inputs["wq"], np.float32)
    bq = np.asarray(inputs["bq"], np.float32)
    wk = np.asarray(inputs["wk"], np.float32)
    wv = np.asarray(inputs["wv"], np.float32)
    bv = np.asarray(inputs["bv"], np.float32)
    gamma = np.asarray(inputs["gamma"], np.float32)

    bf = ml_dtypes.bfloat16
    A = wq.T @ wk                     # (C, C):  A[c, i]
    u = wk.T @ bq                     # (C,)
    a_aug = np.concatenate([A, u[None, :]], axis=0).astype(bf)
    # gamma folded into v: out rows get gamma * v while the appended ones
    # column (softmax denominator) stays unscaled.
    gsc = float(gamma.reshape(-1)[0])
    wvT = (gsc * wv.T).astype(bf)
    wv_dup = np.concatenate([wvT, wvT], axis=0)
    bv8 = np.ascontiguousarray(
        np.tile((gsc * bv).astype(bf)[None, :], (1, 8))
    )

    xf = x.reshape(B, C, N)
    in_maps = []
    for core in range(N_CORES):
        b, h = core // 2, core % 2
        xq = xf[b][:, h * NQ : (h + 1) * NQ]
        xkb = np.ascontiguousarray(xf[b].astype(bf))
        in_maps.append(
            {
                "xk_bf": xkb,
                "xkd_bf": xkb,
                "xq_bf": np.ascontiguousarray(xq.astype(bf)),
                "xq_f32": np.ascontiguousarray(xq),
                "a_aug": a_aug,
                "wv_dup": wv_dup,
                "bv8": bv8,
            }
        )
    return in_maps


_NC_CACHE = None


def kernel(**inputs) -> np.ndarray:
    global _NC_CACHE
    from concourse.bass_utils import run_bass_kernel_spmd

    if _NC_CACHE is None:
        _NC_CACHE = build_nc()
    nc = _NC_CACHE
    in_maps = host_prep(inputs)
    res = run_bass_kernel_spmd(nc, in_maps, core_ids=list(range(N_CORES)))
    x = np.asarray(inputs["x"], np.float32)
    full = np.empty((B, C, N), np.float32)
    for core in range(N_CORES):
        b, h = core // 2, core % 2
        full[b][:, h * NQ : (h + 1) * NQ] = res.results[core]["out"]
    return full.reshape(x.shape)


if __name__ == "__main__":
    rng = np.random.default_rng(0)
    demo = {
        "x": rng.standard_normal((B, C, 16, 16, 16), dtype=np.float32),
        "wq": 0.05 * rng.standard_normal((8, C), dtype=np.float32),
        "bq": 0.05 * rng.standard_normal((8,), dtype=np.float32),
        "wk": 0.05 * rng.standard_normal((8, C), dtype=np.float32),
        "bk": 0.05 * rng.standard_normal((8,), dtype=np.float32),
        "wv": 0.05 * rng.standard_normal((C, C), dtype=np.float32),
        "bv": 0.05 * rng.standard_normal((C,), dtype=np.float32),
        "gamma": np.zeros((1,), np.float32),
    }
    print(kernel(**demo).shape)


# revision 13
# speedup vs baseline: 1.0293x; 1.0293x over previous
"""Trainium2 Bass kernel for nn_PamCell (spatial self-attention, B=4, C=64,
N=16^3=4096, CQ=8) on 8 NeuronCores.

Sharding: core i handles batch i//2 and query-half i%2 (2048 queries vs all
4096 keys). No collectives; host scatters inputs / gathers outputs.

Math: softmax rows are invariant to additive terms that depend only on the
query index, so with A = wq^T wk and u = wk^T bq,
    softmax(q k^T)[n, :] == softmax((A^T x_n + u) . x_m)[n, :]
which turns the QK contraction into a single 64-dim contraction against the
raw input as keys. Energies are in [-5, 5]: the softmax max-subtraction is
skipped (exp cannot overflow).

v2 changes vs the 119.5us baseline (which was ACT-serialized: the scalar
engine ran every exp tile, ~2.2us/granule, while the PE idled):
  - exp is split across engines per granule: ACT does the even key chunk
    (hardware Exp), DVE does the odd chunk with a magic-constant exp
    (bf16 bits of 2^t are linear in t: int16(e*184.665 + 16250.49) bitcast
    to bf16, +-3% sawtooth; RNE convert verified on HW).
  - ~4us of dummy matmuls at the head of the PE queue warm the HAM clock
    gate (2.4GHz) before the first real matmul instead of 15us in.
  - input DMAs spread over 4 engine queues; xq_f32 (epilogue-only) last.
  - bv is pre-loaded into the v^T PSUM accumulation by a K=1 ones matmul,
    dropping the DVE broadcast-add from the prologue.
  - granules run query-half-major so half 0's 1/rowsum (Ln+Exp on ACT, same
    table set) hides under half 1's main loop; the remaining tail is the
    4x [K=1 broadcast matmul -> ACT copy -> DVE mult/add -> DMA] pipeline.
"""

import sys

import numpy as np

try:
    import concourse.bass as bass
except ImportError:  # fresh interpreter without the env paths
    for _p in ("/root/.axon_site", "/root/.axon_site/_ro/trn_rl_repo",
               "/root/.axon_site/_ro/pypackages", "/opt/trn_rl_repo"):
        if _p not in sys.path:
            sys.path.append(_p)
    import concourse.bass as bass

import ml_dtypes

import concourse.tile as tile
from concourse import mybir
from concourse.vector_clock import ScopedClock

BF16 = mybir.dt.bfloat16
F32 = mybir.dt.float32
I16 = mybir.dt.int16
AF = mybir.ActivationFunctionType

B, C, N = 4, 64, 4096
NQ = N // 2          # queries per core
NKC = N // 128       # key chunks of 128
N_CORES = 8
NPAIR = NKC // 2     # 16 key-chunk pairs

S_MAGIC = 128.0 / float(np.log(2.0))   # 184.6650
B_MAGIC = 16256.0 - 5.5078             # RNE-rounded magic bias (HW-verified)
N_WARM = 18                            # dummy warm matmuls, 256 cols each


class _TileContextCompat(tile.TileContext):
    """Split the kernel-tail drain's sem waits across SP instructions;
    this walrus build allows only one sync-wait per CTRL instruction."""

    def _drain_and_barrier(self, tick_clock, wait_clock):
        probe = self.nc.sync.nop()
        wait_clock.add_sem_waits(
            probe.ins, ScopedClock({None: tick_clock.global_clock})
        )
        si = probe.ins.sync_info
        waits = list(si.on_wait) if si is not None else []
        if si is not None:
            probe.ins.sync_info = mybir.SyncInfo(
                on_wait=waits[:1], on_update=list(si.on_update)
            )
        for w in waits[1:]:
            nop = self.nc.sync.nop()
            nop.ins.sync_info = mybir.SyncInfo(on_wait=[w], on_update=[])

        self.nc.sync.drain()
        self.nc.all_engine_barrier()
        assert self.sems is not None
        popped = self.nc._tile_sem_poison_stack.pop()
        assert popped is self._sem_poison
        self.nc.clear_and_free_semaphores(list(self.sems.allocated().values()))
        self.nc.all_engine_barrier()


def _split_sync_waits(nc, max_waits=1):
    """This walrus build rejects instructions carrying more than one sync
    wait; hoist excess waits onto same-engine nops inserted just before."""
    for fn in nc.m.functions:
        for blk in fn.blocks:
            new = []
            changed = False
            for inst in blk.instructions:
                si = inst.sync_info
                if si is not None and si.on_wait and len(si.on_wait) > max_waits:
                    waits = list(si.on_wait)
                    excess = waits[:-max_waits]
                    for i in range(0, len(excess), max_waits):
                        nop = mybir.InstNoOp(
                            name=f"I-{nc.next_id()}-waitsplit", ins=[], outs=[]
                        )
                        nop.engine = inst.engine
                        nop.sync_info = mybir.SyncInfo(
                            on_wait=excess[i : i + max_waits], on_update=[]
                        )
                        new.append(nop)
                    inst.sync_info = mybir.SyncInfo(
                        on_wait=waits[-max_waits:], on_update=list(si.on_update)
                    )
                    changed = True
                new.append(inst)
            if changed:
                blk.instructions = new


def build_nc(split=True):
    nc = bass.Bass(
        "TRN2",
        target_bir_lowering=False,
        debug=False,
        enable_asserts=False,
    )
    xk_bf = nc.dram_tensor("xk_bf", (C, N), BF16, kind="ExternalInput")
    xkd_bf = nc.dram_tensor("xkd_bf", (C, N), BF16, kind="ExternalInput")
    xq_bf = nc.dram_tensor("xq_bf", (C, NQ), BF16, kind="ExternalInput")
    xq_f32 = nc.dram_tensor("xq_f32", (C, NQ), F32, kind="ExternalInput")
    a_aug = nc.dram_tensor("a_aug", (C + 1, C), BF16, kind="ExternalInput")
    wv_dup = nc.dram_tensor("wv_dup", (128, C), BF16, kind="ExternalInput")
    bv8 = nc.dram_tensor("bv8", (1, 512), BF16, kind="ExternalInput")
    scratch = nc.dram_tensor("scratch", (1, NQ), F32, kind="Internal")
    out = nc.dram_tensor("out", (C, NQ), F32, kind="ExternalOutput")

    with _TileContextCompat(nc) as tc:
        with tc.tile_pool(name="consts", bufs=1) as consts:
            # ---- persistent SBUF tensors ----
            xk2 = consts.tile([128, N], BF16, tag="xk2")     # keys, dup rows
            xq = consts.tile([C + 1, NQ], BF16, tag="xq")    # queries + ones
            xqf = consts.tile([C, NQ], F32, tag="xqf")
            a_sb = consts.tile([C + 1, C], BF16, tag="a_sb")
            wv_sb = consts.tile([128, C], BF16, tag="wv_sb")  # wv^T, dup rows
            bv_sb = consts.tile([1, 512], BF16, tag="bv_sb")  # gamma*bv, tiled 8x
            qb2 = consts.tile([128, NQ], BF16, tag="qb2")    # Q, dup rows
            vt = consts.tile([128, NKC, C + 1], BF16, tag="vt")
            ones_kb = consts.tile([1, 128], BF16, tag="ones_kb")
            warm_w = consts.tile([128, 256], BF16, tag="warm_w")
            r_sb = consts.tile([1, NQ], F32, tag="r_sb")
            rb_f = consts.tile([1, NQ], F32, tag="rb_f")
            bc_sb = consts.tile([C, NQ], F32, tag="bc_sb")
            num_sb = consts.tile([C, 1024], F32, tag="num_sb")
            warm_sb = consts.tile([1, 128], F32, tag="warm_sb")

            import bass_rust as _br

            pe_chain = [None]
            act_chain = [None]
            dve_chain = [None]

            def _chained(r, chain, reason="order"):
                if chain[0] is not None:
                    _br.add_dep_helper(r.ins, chain[0].ins, reason=reason)
                chain[0] = r
                return r

            # ---- memsets (vector: tiny consts; gpsimd: larger fills) ----
            nc.vector.memset(ones_kb[:], 1.0)
            nc.gpsimd.memset(warm_w[:], 1.0)
            nc.gpsimd.memset(xq[C : C + 1, :], 1.0)
            nc.gpsimd.memset(vt[:, :, C : C + 1], 1.0)

            # trigger the ~2.7us table load (natural_log set: Ln + Exp + Copy)
            _chained(nc.scalar.activation(warm_sb[:], ones_kb[:], AF.Ln),
                     act_chain)

            # ---- input DMAs (gpsimd SWDGE DMAs are slow: sync/scalar only;
            # each quarter's dup follows its source so energies can start as
            # soon as the first quarter + dup land) ----
            nc.sync.dma_start(xq[:C, :], xq_bf.ap())
            nc.sync.dma_start(xk2[:C, bass.ts(0, N // 4)],
                              xk_bf.ap()[:, bass.ts(0, N // 4)])
            nc.sync.dma_start(xk2[C:, bass.ts(0, N // 4)],
                              xkd_bf.ap()[:, bass.ts(0, N // 4)])
            nc.sync.dma_start(xk2[:C, bass.ts(1, N // 4)],
                              xk_bf.ap()[:, bass.ts(1, N // 4)])
            nc.sync.dma_start(xk2[C:, bass.ts(1, N // 4)],
                              xkd_bf.ap()[:, bass.ts(1, N // 4)])
            nc.scalar.dma_start(a_sb[:], a_aug.ap())
            nc.scalar.dma_start(wv_sb[:], wv_dup.ap())
            nc.scalar.dma_start(bv_sb[:], bv8.ap())
            nc.scalar.dma_start(xk2[:C, bass.ts(2, N // 4)],
                                xk_bf.ap()[:, bass.ts(2, N // 4)])
            nc.scalar.dma_start(xk2[C:, bass.ts(2, N // 4)],
                                xkd_bf.ap()[:, bass.ts(2, N // 4)])
            nc.scalar.dma_start(xk2[:C, bass.ts(3, N // 4)],
                                xk_bf.ap()[:, bass.ts(3, N // 4)])
            nc.scalar.dma_start(xk2[C:, bass.ts(3, N // 4)],
                                xkd_bf.ap()[:, bass.ts(3, N // 4)])
            # epilogue-only fp32 queries last
            nc.scalar.dma_start(xqf[:], xq_f32.ap())

            # ---- prologue ----
            # PSUM: q0(2) q1(2) vp(2) warm(1) = 7 banks; released before the
            # main loop so e0/e1/out_big can take all 8.
            with tc.tile_pool(name="psum_pro", bufs=1, space="PSUM") as pro:
                # dummy matmuls: keep the PE busy from t~0 so the HAM clock
                # gate is warm (2.4GHz) when the first real matmul issues.
                # single row group so consecutive warm MMs serialize: two
                # row-tiled MMs writing the same PSUM region would race.
                warm_ps = pro.tile([128, 256], F32, tag="warm_ps")
                for i in range(N_WARM):
                    _chained(nc.tensor.matmul(
                        warm_ps[:, :],
                        warm_w[0:64, 0:128],
                        warm_w[0:64, :],
                        start=True,
                        stop=True,
                        tile_position=(0, 0),
                        skip_group_check=True,
                    ), pe_chain)

                # Q = a_aug^T xq_aug, written twice (col-tiled) so both
                # partition halves hold a copy for the row-tiled energy MMs
                q_halves = [
                    pro.tile([128, NQ // 2], F32, tag=f"q{h}", name=f"q{h}")
                    for h in range(2)
                ]
                for j in range(NQ // 512):
                    q_ps = q_halves[j // 2]
                    js = bass.ts(j % 2, 512)
                    _chained(nc.tensor.matmul(
                        q_ps[:C, js],
                        a_sb[:],
                        xq[:, bass.ts(j, 512)],
                        start=True,
                        stop=True,
                        tile_position=(0, 0),
                    ), pe_chain)
                    _chained(nc.tensor.matmul(
                        q_ps[C:, js],
                        a_sb[:],
                        xq[:, bass.ts(j, 512)],
                        start=True,
                        stop=True,
                        tile_position=(0, 64),
                    ), pe_chain)
                # Q scaled by 128/ln2 here so the DVE magic exp is a single
                # add; the ACT exps undo it with their free scale field.
                for h in range(2):
                    _chained(nc.vector.tensor_scalar_mul(
                        qb2[:, bass.ts(h, NQ // 2)], q_halves[h][:], S_MAGIC
                    ), dve_chain)

                # v^T per key chunk, row-tiled pairs; bv is pre-loaded into
                # the accumulation by a K=1 ones matmul so no bias add is
                # needed afterwards. NOT pe-chained: the scheduler slots
                # these into PE gaps while the first exps run.
                vt_r = vt.rearrange("p (t two) c -> p t two c", two=2)
                for g in range(2):
                    vp = pro.tile([128, 1024], F32, tag="vp", bufs=1, name="vp")
                    for half in range(2):
                        nc.tensor.matmul(
                            vp[:, bass.ts(half, 512)],
                            ones_kb[:, :],
                            bv_sb[:, :],
                            start=True,
                            stop=False,
                            skip_group_check=True,
                        )
                    for t in range(8):
                        pair = 8 * g + t
                        nc.tensor.matmul(
                            vp[:, bass.ts(t, C)],
                            xk2[:C, bass.ts(2 * pair, 128)],
                            wv_sb[:C, :],
                            start=False,
                            stop=True,
                            tile_position=(0, 0),
                            skip_group_check=True,
                        )
                        nc.tensor.matmul(
                            vp[:, bass.ds(512 + t * C, C)],
                            xk2[C:, bass.ts(2 * pair + 1, 128)],
                            wv_sb[C:, :],
                            start=False,
                            stop=True,
                            tile_position=(64, 0),
                            skip_group_check=True,
                        )
                    for half in range(2):
                        _chained(nc.scalar.copy(
                            vt_r[:, bass.ts(g, 8), half, :C],
                            vp[:, bass.ts(half, 512)].rearrange(
                                "p (t c) -> p t c", t=8
                            ),
                        ), act_chain)

            # ---- main loop (query-half-major) ----
            with (
                tc.tile_pool(name="psum_e", bufs=1, space="PSUM") as pe_pool,
                tc.tile_pool(name="psum_out", bufs=1, space="PSUM") as pout,
            ):
                out_big = pout.tile([C + 1, NQ], F32, tag="out_big",
                                    name="out_big")
                out_ps = [
                    out_big[:, bass.ts(qg, 512)] for qg in range(NQ // 512)
                ]
                with (
                    tc.tile_pool(name="pt_pool", bufs=6) as pt_pool,
                ):
                    # granule = (pair, qh), qh-major: granules 0-15 cover
                    # query half 0 over all key pairs, 16-31 half 1. Half 0's
                    # epilogue recip overlaps half 1's compute.
                    NG = NKC  # 32 granules
                    gr = [(pair, qh) for qh in range(2) for pair in range(NPAIR)]

                    def energies(g):
                        """All 4 energy MMs of granule g, interleaved h0/h64
                        so adjacent MMs run concurrently in disjoint row
                        groups."""
                        pair, qh = gr[g]
                        qoff = qh * 1024
                        es = []
                        for half in range(2):
                            es.append(pe_pool.tile(
                                [128, 1024], F32, tag=f"e{half}",
                                name=f"e{half}"
                            ))
                        for j in range(2):
                            for half in range(2):
                                mc = 2 * pair + half
                                lo = C * half
                                _chained(
                                    nc.tensor.matmul(
                                        es[half][:, bass.ts(j, 512)],
                                        xk2[lo : lo + C, bass.ts(mc, 128)],
                                        qb2[lo : lo + C,
                                            bass.ds(qoff + j * 512, 512)],
                                        start=True,
                                        stop=True,
                                        tile_position=(lo, 0),
                                    ),
                                    pe_chain,
                                    "pe-order",
                                )
                        return es

                    def do_exp_act(e):
                        """Even chunk: hardware Exp on the scalar engine.
                        Energies carry the 128/ln2 magic scale; undo it with
                        the instruction's free affine scale."""
                        pt = pt_pool.tile([128, 1024], BF16, tag="pt0",
                                          name="pt0")
                        _chained(
                            nc.scalar.activation(pt[:], e[:], AF.Exp,
                                                 scale=1.0 / S_MAGIC),
                            act_chain,
                            "act-order",
                        )
                        return pt

                    def do_exp_dve(e):
                        """Odd chunk: magic-constant exp on the vector
                        engine: bf16 bits of 2^(e/ln2) are int16(e_s + b)
                        with e_s pre-scaled in qb2."""
                        pt = pt_pool.tile([128, 1024], BF16, tag="pt1",
                                          name="pt1")
                        _chained(
                            nc.vector.tensor_scalar_add(
                                pt[:].bitcast(I16), e[:], B_MAGIC,
                            ),
                            dve_chain,
                            "dve-order",
                        )
                        return pt

                    def outs(g, half, pt):
                        pair, qh = gr[g]
                        mc = 2 * pair + half
                        for j in range(2):
                            qg = 2 * qh + j
                            _chained(
                                nc.tensor.matmul(
                                    out_ps[qg][:],
                                    vt[:, mc, :],
                                    pt[:, bass.ts(j, 512)],
                                    start=(pair == 0),
                                    stop=(pair == NPAIR - 1),
                                    skip_group_check=True,
                                ),
                                pe_chain,
                                "pe-order",
                            )

                    def recip_chunk(c, width):
                        """1/rowsum for queries [c*width, (c+1)*width): Ln
                        then Exp(-x) on ACT (same natural_log table set as
                        the main exps). Chunked so single ops never dam up
                        the ACT queue ahead of the main-loop exps."""
                        cs = bass.ts(c, width)
                        _chained(nc.scalar.activation(
                            r_sb[:, cs], out_big[C : C + 1, cs], AF.Ln
                        ), act_chain, "act-order")
                        _chained(nc.scalar.activation(
                            rb_f[:, cs], r_sb[:, cs], AF.Exp, scale=-1.0
                        ), act_chain, "act-order")

                    def bcast(qg):
                        """Broadcast 1/rowsum across 64 partitions by a DRAM
                        round trip: row out, stride-0 read back. Both DMAs on
                        the sync queue, so they execute in order."""
                        qs = bass.ts(qg, 512)
                        nc.sync.dma_start(scratch.ap()[:, qs], rb_f[:, qs])
                        nc.sync.dma_start(
                            bc_sb[:, qs],
                            scratch.ap()[:, qs].broadcast_to((C, 512)),
                        )

                    # half-0 epilogue steps, one per half-1 granule: recip
                    # in [1,256] chunks, then broadcast + gpsimd divide/add
                    # (ACT stages the PSUM numerator; gpsimd is otherwise
                    # idle and DVE is the granule clock).
                    def epi0_step(k):
                        if k < 8:
                            if k % 2 == 0:
                                recip_chunk(k // 2, 256)
                            return
                        if k == 8:
                            bcast(0)
                            bcast(1)
                            return
                        if k in (9, 10):
                            qg = k - 9
                            _chained(nc.scalar.copy(
                                num_sb[:, bass.ts(qg, 512)],
                                out_ps[qg][:C, :],
                            ), act_chain, "act-order")
                            return
                        if k in (11, 13):
                            qg = (k - 11) // 2
                            qs = bass.ts(qg, 512)
                            nc.gpsimd.tensor_tensor(
                                num_sb[:, qs], num_sb[:, qs], bc_sb[:, qs],
                                mybir.AluOpType.mult,
                            )
                            return
                        if k in (12, 14):
                            qg = (k - 12) // 2
                            qs = bass.ts(qg, 512)
                            nc.gpsimd.tensor_tensor(
                                num_sb[:, qs], num_sb[:, qs], xqf[:, qs],
                                mybir.AluOpType.add,
                            )
                            nc.sync.dma_start(out.ap()[:, qs],
                                              num_sb[:, qs])
                            return

                    es = {0: energies(0)}
                    for g in range(NG):
                        eA, eB = es.pop(g)
                        ptA = do_exp_act(eA)
                        ptB = do_exp_dve(eB)
                        if g + 1 < NG:
                            es[g + 1] = energies(g + 1)
                        outs(g, 0, ptA)
                        outs(g, 1, ptB)
                        if g >= NPAIR:
                            epi0_step(g - NPAIR)

                # ---- tail: half-1 epilogue on DVE (reads the numerator
                # straight from PSUM), pipelined per query group ----
                with tc.tile_pool(name="epi", bufs=2) as epi:
                    for qg in range(2, 4):
                        recip_chunk(2 * qg, 256)
                        recip_chunk(2 * qg + 1, 256)
                        bcast(qg)
                        qs = bass.ts(qg, 512)
                        t_sb = epi.tile([C, 512], F32, tag="t_sb")
                        _chained(nc.vector.tensor_tensor(
                            t_sb[:], out_ps[qg][:C, :], bc_sb[:, qs],
                            mybir.AluOpType.mult,
                        ), dve_chain, "dve-order")
                        _chained(nc.vector.tensor_tensor(
                            t_sb[:], t_sb[:], xqf[:, qs],
                            mybir.AluOpType.add,
                        ), dve_chain, "dve-order")
                        nc.sync.dma_start(out.ap()[:, qs], t_sb[:])

    if split:
        _split_sync_waits(nc)
    return nc


def host_prep(inputs):
    """Full inputs -> list of 8 per-core input maps."""
    x = np.asarray(inputs["x"], np.float32)
    wq = np.asarray(inputs["wq"], np.float32)
    bq = np.asarray(inputs["bq"], np.float32)
    wk = np.asarray(inputs["wk"], np.float32)
    wv = np.asarray(inputs["wv"], np.float32)
    bv = np.asarray(inputs["bv"], np.float32)
    gamma = np.asarray(inputs["gamma"], np.float32)

    bf = ml_dtypes.bfloat16
    A = wq.T @ wk                     # (C, C):  A[c, i]
    u = wk.T @ bq                     # (C,)
    a_aug = np.concatenate([A, u[None, :]], axis=0).astype(bf)
    # gamma folded into v: out rows get gamma * v while the appended ones
    # column (softmax denominator) stays unscaled.
    gsc = float(gamma.reshape(-1)[0])
    wvT = (gsc * wv.T).astype(bf)
    wv_dup = np.concatenate([wvT, wvT], axis=0)
    bv8 = np.ascontiguousarray(
        np.tile((gsc * bv).astype(bf)[None, :], (1, 8))
    )

    xf = x.reshape(B, C, N)
    in_maps = []
    for core in range(N_CORES):
        b, h = core // 2, core % 2
        xq = xf[b][:, h * NQ : (h + 1) * NQ]
        xkb = np.ascontiguousarray(xf[b].astype(bf))
        in_maps.append(
            {
                "xk_bf": xkb,
                "xkd_bf": xkb,
                "xq_bf": np.ascontiguousarray(xq.astype(bf)),
                "xq_f32": np.ascontiguousarray(xq),
                "a_aug": a_aug,
                "wv_dup": wv_dup,
                "bv8": bv8,
            }
        )
    return in_maps


_NC_CACHE = None


def kernel(**inputs) -> np.ndarray:
    global _NC_CACHE
    from concourse.bass_utils import run_bass_kernel_spmd

    if _NC_CACHE is None:
        _NC_CACHE = build_nc()
    nc = _NC_CACHE
    in_maps = host_prep(inputs)
    res = run_bass_kernel_spmd(nc, in_maps, core_ids=list(range(N_CORES)))
    x = np.asarray(inputs["x"], np.float32)
    full = np.empty((B, C, N), np.float32)
    for core in range(N_CORES):
        b, h = core // 2, core % 2
        full[b][:, h * NQ : (h + 1) * NQ] = res.results[core]["out"]
    return full.reshape(x.shape)


if __name__ == "__main__":
    rng = np.random.default_rng(0)
    demo = {
        "x": rng.standard_normal((B, C, 16, 16, 16), dtype=np.float32),
        "wq": 0.05 * rng.standard_normal((8, C), dtype=np.float32),
        "bq": 0.05 * rng.standard_normal((8,), dtype=np.float32),
        "wk": 0.05 * rng.standard_normal((8, C), dtype=np.float32),
        "bk": 0.05 * rng.standard_normal((8,), dtype=np.float32),
        "wv": 0.05 * rng.standard_normal((C, C), dtype=np.float32),
        "bv": 0.05 * rng.standard_normal((C,), dtype=np.float32),
        "gamma": np.zeros((1,), np.float32),
    }
    print(kernel(**demo).shape)


# revision 19
# speedup vs baseline: 1.1766x; 1.1430x over previous
"""Trainium2 Bass kernel for nn_PamCell (spatial self-attention, B=4, C=64,
N=16^3=4096, CQ=8) on 8 NeuronCores.

Sharding: core i handles batch i//2 and query-half i%2 (2048 queries vs all
4096 keys). No collectives; host scatters inputs / gathers outputs.

Math: softmax rows are invariant to additive terms that depend only on the
query index, so with A = wq^T wk and u = wk^T bq,
    softmax(q k^T)[n, :] == softmax((A^T x_n + u) . x_m)[n, :]
which turns the QK contraction into a single 64-dim contraction against the
raw input as keys. Energies are in [-5, 5]: the softmax max-subtraction is
skipped (exp cannot overflow).

v2 changes vs the 119.5us baseline (which was ACT-serialized: the scalar
engine ran every exp tile, ~2.2us/granule, while the PE idled):
  - exp is split across engines per granule: ACT does the even key chunk
    (hardware Exp), DVE does the odd chunk with a magic-constant exp
    (bf16 bits of 2^t are linear in t: int16(e*184.665 + 16250.49) bitcast
    to bf16, +-3% sawtooth; RNE convert verified on HW).
  - ~4us of dummy matmuls at the head of the PE queue warm the HAM clock
    gate (2.4GHz) before the first real matmul instead of 15us in.
  - input DMAs spread over 4 engine queues; xq_f32 (epilogue-only) last.
  - bv is pre-loaded into the v^T PSUM accumulation by a K=1 ones matmul,
    dropping the DVE broadcast-add from the prologue.
  - granules run query-half-major so half 0's 1/rowsum (Ln+Exp on ACT, same
    table set) hides under half 1's main loop; the remaining tail is the
    4x [K=1 broadcast matmul -> ACT copy -> DVE mult/add -> DMA] pipeline.
"""

import sys

import numpy as np

try:
    import concourse.bass as bass
except ImportError:  # fresh interpreter without the env paths
    for _p in ("/root/.axon_site", "/root/.axon_site/_ro/trn_rl_repo",
               "/root/.axon_site/_ro/pypackages", "/opt/trn_rl_repo"):
        if _p not in sys.path:
            sys.path.append(_p)
    import concourse.bass as bass

import ml_dtypes

import concourse.tile as tile
from concourse import mybir
from concourse.vector_clock import ScopedClock

BF16 = mybir.dt.bfloat16
F32 = mybir.dt.float32
I16 = mybir.dt.int16
AF = mybir.ActivationFunctionType

B, C, N = 4, 64, 4096
NQ = N // 2          # queries per core
NKC = N // 128       # key chunks of 128
N_CORES = 8
NPAIR = NKC // 2     # 16 key-chunk pairs

S_MAGIC = 128.0 / float(np.log(2.0))   # 184.6650
B_MAGIC = 16256.0 - 5.5078             # RNE-rounded magic bias (HW-verified)
N_WARM = 14                            # dummy warm matmuls, 512 cols each


class _TileContextCompat(tile.TileContext):
    """Split the kernel-tail drain's sem waits across SP instructions;
    this walrus build allows only one sync-wait per CTRL instruction."""

    def _drain_and_barrier(self, tick_clock, wait_clock):
        probe = self.nc.sync.nop()
        wait_clock.add_sem_waits(
            probe.ins, ScopedClock({None: tick_clock.global_clock})
        )
        si = probe.ins.sync_info
        waits = list(si.on_wait) if si is not None else []
        if si is not None:
            probe.ins.sync_info = mybir.SyncInfo(
                on_wait=waits[:1], on_update=list(si.on_update)
            )
        for w in waits[1:]:
            nop = self.nc.sync.nop()
            nop.ins.sync_info = mybir.SyncInfo(on_wait=[w], on_update=[])

        self.nc.sync.drain()
        self.nc.all_engine_barrier()
        assert self.sems is not None
        popped = self.nc._tile_sem_poison_stack.pop()
        assert popped is self._sem_poison
        self.nc.clear_and_free_semaphores(list(self.sems.allocated().values()))
        self.nc.all_engine_barrier()


def _split_sync_waits(nc, max_waits=1):
    """This walrus build rejects instructions carrying more than one sync
    wait; hoist excess waits onto same-engine nops inserted just before."""
    for fn in nc.m.functions:
        for blk in fn.blocks:
            new = []
            changed = False
            for inst in blk.instructions:
                si = inst.sync_info
                if si is not None and si.on_wait and len(si.on_wait) > max_waits:
                    waits = list(si.on_wait)
                    excess = waits[:-max_waits]
                    for i in range(0, len(excess), max_waits):
                        nop = mybir.InstNoOp(
                            name=f"I-{nc.next_id()}-waitsplit", ins=[], outs=[]
                        )
                        nop.engine = inst.engine
                        nop.sync_info = mybir.SyncInfo(
                            on_wait=excess[i : i + max_waits], on_update=[]
                        )
                        new.append(nop)
                    inst.sync_info = mybir.SyncInfo(
                        on_wait=waits[-max_waits:], on_update=list(si.on_update)
                    )
                    changed = True
                new.append(inst)
            if changed:
                blk.instructions = new


def build_nc(split=True):
    nc = bass.Bass(
        "TRN2",
        target_bir_lowering=False,
        debug=False,
        enable_asserts=False,
    )
    xk_bf = nc.dram_tensor("xk_bf", (C, N), BF16, kind="ExternalInput")
    xkd_bf = nc.dram_tensor("xkd_bf", (C, N), BF16, kind="ExternalInput")
    xq_bf = nc.dram_tensor("xq_bf", (C, NQ), BF16, kind="ExternalInput")
    xq_f32 = nc.dram_tensor("xq_f32", (C, NQ), F32, kind="ExternalInput")
    a_aug = nc.dram_tensor("a_aug", (C + 1, C), BF16, kind="ExternalInput")
    wv_dup = nc.dram_tensor("wv_dup", (128, C), BF16, kind="ExternalInput")
    bv8 = nc.dram_tensor("bv8", (1, 512), BF16, kind="ExternalInput")
    scratch = nc.dram_tensor("scratch", (1, NQ), F32, kind="Internal")
    out = nc.dram_tensor("out", (C, NQ), F32, kind="ExternalOutput")

    with _TileContextCompat(nc) as tc:
        with tc.tile_pool(name="consts", bufs=1) as consts:
            # ---- persistent SBUF tensors ----
            xk2 = consts.tile([128, N], BF16, tag="xk2")     # keys, dup rows
            xq = consts.tile([C + 1, NQ], BF16, tag="xq")    # queries + ones
            xqf = consts.tile([C, NQ], F32, tag="xqf")
            a_sb = consts.tile([C + 1, C], BF16, tag="a_sb")
            wv_sb = consts.tile([128, C], BF16, tag="wv_sb")  # wv^T, dup rows
            bv_sb = consts.tile([1, 512], BF16, tag="bv_sb")  # gamma*bv, tiled 8x
            qb2 = consts.tile([128, NQ], BF16, tag="qb2")    # Q, dup rows
            vt = consts.tile([128, NKC, C + 1], BF16, tag="vt")
            ones_kb = consts.tile([1, 128], BF16, tag="ones_kb")
            warm_w = consts.tile([128, 512], BF16, tag="warm_w")
            r_sb = consts.tile([1, NQ], F32, tag="r_sb")
            rb_f = consts.tile([1, NQ], F32, tag="rb_f")
            rb_b = consts.tile([1, NQ], BF16, tag="rb_b")
            bc_sb = consts.tile([C, NQ], F32, tag="bc_sb")
            num_sb = consts.tile([C, 1024], F32, tag="num_sb")
            warm_sb = consts.tile([1, 128], F32, tag="warm_sb")

            import bass_rust as _br

            pe_chain = [None]
            act_chain = [None]
            dve_chain = [None]

            def _chained(r, chain, reason="order"):
                if chain[0] is not None:
                    _br.add_dep_helper(r.ins, chain[0].ins, reason=reason)
                chain[0] = r
                return r

            # ---- memsets (vector: tiny consts; gpsimd: larger fills) ----
            nc.vector.memset(ones_kb[:], 1.0)
            nc.gpsimd.memset(warm_w[:], 1.0)
            nc.gpsimd.memset(xq[C : C + 1, :], 1.0)
            nc.gpsimd.memset(vt[:, :, C : C + 1], 1.0)

            # trigger the ~2.7us table load (natural_log set: Ln + Exp + Copy)
            _chained(nc.scalar.activation(warm_sb[:], ones_kb[:], AF.Ln),
                     act_chain)

            # ---- input DMAs (gpsimd SWDGE DMAs are slow: sync/scalar only;
            # each quarter's dup follows its source so energies can start as
            # soon as the first quarter + dup land) ----
            nc.sync.dma_start(xq[:C, :], xq_bf.ap())
            nc.sync.dma_start(xk2[:C, bass.ts(0, N // 4)],
                              xk_bf.ap()[:, bass.ts(0, N // 4)])
            nc.sync.dma_start(xk2[C:, bass.ts(0, N // 4)],
                              xkd_bf.ap()[:, bass.ts(0, N // 4)])
            nc.sync.dma_start(xk2[:C, bass.ts(1, N // 4)],
                              xk_bf.ap()[:, bass.ts(1, N // 4)])
            nc.sync.dma_start(xk2[C:, bass.ts(1, N // 4)],
                              xkd_bf.ap()[:, bass.ts(1, N // 4)])
            nc.scalar.dma_start(a_sb[:], a_aug.ap())
            nc.scalar.dma_start(wv_sb[:], wv_dup.ap())
            nc.scalar.dma_start(bv_sb[:], bv8.ap())
            nc.scalar.dma_start(xk2[:C, bass.ts(2, N // 4)],
                                xk_bf.ap()[:, bass.ts(2, N // 4)])
            nc.scalar.dma_start(xk2[C:, bass.ts(2, N // 4)],
                                xkd_bf.ap()[:, bass.ts(2, N // 4)])
            nc.scalar.dma_start(xk2[:C, bass.ts(3, N // 4)],
                                xk_bf.ap()[:, bass.ts(3, N // 4)])
            nc.scalar.dma_start(xk2[C:, bass.ts(3, N // 4)],
                                xkd_bf.ap()[:, bass.ts(3, N // 4)])
            # epilogue-only fp32 queries last
            nc.scalar.dma_start(xqf[:], xq_f32.ap())

            # ---- prologue ----
            # PSUM: q0(2) q1(2) vp(2) warm(1) = 7 banks; released before the
            # main loop so e0/e1/out_big can take all 8.
            with tc.tile_pool(name="psum_pro", bufs=1, space="PSUM") as pro:
                # dummy matmuls: keep the PE busy from t~0 so the HAM clock
                # gate flips to 2.4GHz before the first real matmul. The HAM
                # needs a fully-busy 3.4us window, so the stream must be
                # dense: alternate row groups into two separate PSUM tiles
                # (same tile would be a concurrent-writer race).
                warm_ps = [
                    pro.tile([128, 512], F32, tag=f"warm_ps{h}",
                             name=f"warm_ps{h}")
                    for h in range(2)
                ]
                for i in range(N_WARM):
                    lo = 64 * (i % 2)
                    _chained(nc.tensor.matmul(
                        warm_ps[i % 2][:, :],
                        warm_w[lo : lo + 64, 0:128],
                        warm_w[lo : lo + 64, :],
                        start=True,
                        stop=True,
                        tile_position=(lo, 0),
                        skip_group_check=True,
                    ), pe_chain)

                # Q = a_aug^T xq_aug, written twice (col-tiled) so both
                # partition halves hold a copy for the row-tiled energy MMs
                q_halves = [
                    pro.tile([128, NQ // 2], F32, tag=f"q{h}", name=f"q{h}")
                    for h in range(2)
                ]
                for j in range(NQ // 512):
                    q_ps = q_halves[j // 2]
                    js = bass.ts(j % 2, 512)
                    _chained(nc.tensor.matmul(
                        q_ps[:C, js],
                        a_sb[:],
                        xq[:, bass.ts(j, 512)],
                        start=True,
                        stop=True,
                        tile_position=(0, 0),
                    ), pe_chain)
                    _chained(nc.tensor.matmul(
                        q_ps[C:, js],
                        a_sb[:],
                        xq[:, bass.ts(j, 512)],
                        start=True,
                        stop=True,
                        tile_position=(0, 64),
                    ), pe_chain)
                # Q scaled by 128/ln2 here so the DVE magic exp is a single
                # add; the ACT exps undo it with their free scale field.
                for h in range(2):
                    _chained(nc.vector.tensor_scalar_mul(
                        qb2[:, bass.ts(h, NQ // 2)], q_halves[h][:], S_MAGIC
                    ), dve_chain)

                # v^T per key chunk, row-tiled pairs; bv is pre-loaded into
                # the accumulation by a K=1 ones matmul so no bias add is
                # needed afterwards. NOT pe-chained: the scheduler slots
                # these into PE gaps while the first exps run.
                vt_r = vt.rearrange("p (t two) c -> p t two c", two=2)
                for g in range(2):
                    vp = pro.tile([128, 1024], F32, tag="vp", bufs=1, name="vp")
                    for half in range(2):
                        nc.tensor.matmul(
                            vp[:, bass.ts(half, 512)],
                            ones_kb[:, :],
                            bv_sb[:, :],
                            start=True,
                            stop=False,
                            skip_group_check=True,
                        )
                    for t in range(8):
                        pair = 8 * g + t
                        nc.tensor.matmul(
                            vp[:, bass.ts(t, C)],
                            xk2[:C, bass.ts(2 * pair, 128)],
                            wv_sb[:C, :],
                            start=False,
                            stop=True,
                            tile_position=(0, 0),
                            skip_group_check=True,
                        )
                        nc.tensor.matmul(
                            vp[:, bass.ds(512 + t * C, C)],
                            xk2[C:, bass.ts(2 * pair + 1, 128)],
                            wv_sb[C:, :],
                            start=False,
                            stop=True,
                            tile_position=(64, 0),
                            skip_group_check=True,
                        )
                    for half in range(2):
                        _chained(nc.scalar.copy(
                            vt_r[:, bass.ts(g, 8), half, :C],
                            vp[:, bass.ts(half, 512)].rearrange(
                                "p (t c) -> p t c", t=8
                            ),
                        ), act_chain)

            # ---- main loop (query-half-major) ----
            with (
                tc.tile_pool(name="psum_e", bufs=1, space="PSUM") as pe_pool,
                tc.tile_pool(name="psum_out", bufs=1, space="PSUM") as pout,
            ):
                out_big = pout.tile([C + 1, NQ], F32, tag="out_big",
                                    name="out_big")
                out_ps = [
                    out_big[:, bass.ts(qg, 512)] for qg in range(NQ // 512)
                ]
                with (
                    tc.tile_pool(name="pt_pool", bufs=6) as pt_pool,
                ):
                    # granule = (pair, qh), qh-major: granules 0-15 cover
                    # query half 0 over all key pairs, 16-31 half 1. Half 0's
                    # epilogue recip overlaps half 1's compute.
                    NG = NKC  # 32 granules
                    gr = [(pair, qh) for qh in range(2) for pair in range(NPAIR)]

                    def energies(g):
                        """All 4 energy MMs of granule g, one single-bank
                        [128,512] PSUM tile per (half, j): interleaved h0/h64
                        so adjacent MMs run concurrently in disjoint row
                        groups, and each tile is released to the exp engines
                        after a single MM so the energy->exp->bank-free
                        recurrence stays off the critical path."""
                        pair, qh = gr[g]
                        qoff = qh * 1024
                        es = [[None, None], [None, None]]
                        for j in range(2):
                            for half in range(2):
                                es[half][j] = pe_pool.tile(
                                    [128, 512], F32, tag=f"e{half}{j}",
                                    name=f"e{half}{j}"
                                )
                                mc = 2 * pair + half
                                lo = C * half
                                _chained(
                                    nc.tensor.matmul(
                                        es[half][j][:],
                                        xk2[lo : lo + C, bass.ts(mc, 128)],
                                        qb2[lo : lo + C,
                                            bass.ds(qoff + j * 512, 512)],
                                        start=True,
                                        stop=True,
                                        tile_position=(lo, 0),
                                    ),
                                    pe_chain,
                                    "pe-order",
                                )
                        return es

                    def do_exp_act(e2):
                        """Even chunk: hardware Exp on the scalar engine.
                        Energies carry the 128/ln2 magic scale; undo it with
                        the instruction's free affine scale."""
                        pt = pt_pool.tile([128, 1024], BF16, tag="pt0",
                                          name="pt0")
                        for j in range(2):
                            _chained(
                                nc.scalar.activation(
                                    pt[:, bass.ts(j, 512)], e2[j][:], AF.Exp,
                                    scale=1.0 / S_MAGIC),
                                act_chain,
                                "act-order",
                            )
                        return pt

                    def do_exp_dve(e2):
                        """Odd chunk: magic-constant exp on the vector
                        engine: bf16 bits of 2^(e/ln2) are int16(e_s + b)
                        with e_s pre-scaled in qb2."""
                        pt = pt_pool.tile([128, 1024], BF16, tag="pt1",
                                          name="pt1")
                        for j in range(2):
                            _chained(
                                nc.vector.tensor_scalar_add(
                                    pt[:, bass.ts(j, 512)].bitcast(I16),
                                    e2[j][:], B_MAGIC,
                                ),
                                dve_chain,
                                "dve-order",
                            )
                        return pt

                    def outs(g, half, pt):
                        pair, qh = gr[g]
                        mc = 2 * pair + half
                        for j in range(2):
                            qg = 2 * qh + j
                            _chained(
                                nc.tensor.matmul(
                                    out_ps[qg][:],
                                    vt[:, mc, :],
                                    pt[:, bass.ts(j, 512)],
                                    start=(pair == 0),
                                    stop=(pair == NPAIR - 1),
                                    skip_group_check=True,
                                ),
                                pe_chain,
                                "pe-order",
                            )

                    def recip_chunk(c, width):
                        """1/rowsum for queries [c*width, (c+1)*width): Ln
                        then Exp(-x) on ACT (same natural_log table set as
                        the main exps). Chunked so single ops never dam up
                        the ACT queue ahead of the main-loop exps."""
                        cs = bass.ts(c, width)
                        _chained(nc.scalar.activation(
                            r_sb[:, cs], out_big[C : C + 1, cs], AF.Ln
                        ), act_chain, "act-order")
                        _chained(nc.scalar.activation(
                            rb_f[:, cs], r_sb[:, cs], AF.Exp, scale=-1.0
                        ), act_chain, "act-order")

                    def bcast(qg):
                        """Broadcast 1/rowsum across 64 partitions by a DRAM
                        round trip: row out, stride-0 read back. Both DMAs on
                        the sync queue, so they execute in order."""
                        qs = bass.ts(qg, 512)
                        nc.sync.dma_start(scratch.ap()[:, qs], rb_f[:, qs])
                        nc.sync.dma_start(
                            bc_sb[:, qs],
                            scratch.ap()[:, qs].broadcast_to((C, 512)),
                        )

                    # half-0 epilogue steps, one per half-1 granule: recip
                    # in [1,256] chunks, then broadcast + gpsimd divide/add
                    # (ACT stages the PSUM numerator; gpsimd is otherwise
                    # idle and DVE is the granule clock).
                    def epi0_step(k):
                        if k < 8:
                            if k % 2 == 0:
                                recip_chunk(k // 2, 256)
                            return
                        if k == 8:
                            bcast(0)
                            bcast(1)
                            return
                        if k in (9, 10):
                            qg = k - 9
                            _chained(nc.scalar.copy(
                                num_sb[:, bass.ts(qg, 512)],
                                out_ps[qg][:C, :],
                            ), act_chain, "act-order")
                            return
                        if k in (11, 13):
                            qg = (k - 11) // 2
                            qs = bass.ts(qg, 512)
                            nc.gpsimd.tensor_tensor(
                                num_sb[:, qs], num_sb[:, qs], bc_sb[:, qs],
                                mybir.AluOpType.mult,
                            )
                            return
                        if k in (12, 14):
                            qg = (k - 12) // 2
                            qs = bass.ts(qg, 512)
                            nc.gpsimd.tensor_tensor(
                                num_sb[:, qs], num_sb[:, qs], xqf[:, qs],
                                mybir.AluOpType.add,
                            )
                            nc.sync.dma_start(out.ap()[:, qs],
                                              num_sb[:, qs])
                            return

                    es = {0: energies(0)}
                    for g in range(NG):
                        eA2, eB2 = es.pop(g)
                        ptA = do_exp_act(eA2)
                        ptB = do_exp_dve(eB2)
                        if g + 1 < NG:
                            es[g + 1] = energies(g + 1)
                        outs(g, 0, ptA)
                        outs(g, 1, ptB)
                        if g >= NPAIR:
                            epi0_step(g - NPAIR)

                # ---- tail: half-1 epilogue, pipelined per query group.
                # The DRAM round-trip broadcast costs ~4.7us serial, so here
                # (nothing left to hide under) broadcast via K=1 ones matmul
                # into the freed energy banks + ACT copy instead; the recip
                # Exp writes bf16 directly for the matmul rhs. ----
                with tc.tile_pool(name="epi", bufs=2) as epi:
                    for qg in range(2, 4):
                        qs = bass.ts(qg, 512)
                        _chained(nc.scalar.activation(
                            r_sb[:, qs], out_big[C : C + 1, qs], AF.Ln
                        ), act_chain, "act-order")
                        _chained(nc.scalar.activation(
                            rb_b[:, qs], r_sb[:, qs], AF.Exp, scale=-1.0
                        ), act_chain, "act-order")
                        bc_ps = pe_pool.tile(
                            [C, 512], F32, tag=f"e{qg % 2}0",
                            name=f"bc{qg}"
                        )
                        _chained(nc.tensor.matmul(
                            bc_ps[:],
                            ones_kb[:, :C],
                            rb_b[:, qs],
                            start=True,
                            stop=True,
                        ), pe_chain, "pe-order")
                        _chained(nc.scalar.copy(
                            bc_sb[:, qs], bc_ps[:]
                        ), act_chain, "act-order")
                        t_sb = epi.tile([C, 512], F32, tag="t_sb")
                        _chained(nc.vector.tensor_tensor(
                            t_sb[:], out_ps[qg][:C, :], bc_sb[:, qs],
                            mybir.AluOpType.mult,
                        ), dve_chain, "dve-order")
                        _chained(nc.vector.tensor_tensor(
                            t_sb[:], t_sb[:], xqf[:, qs],
                            mybir.AluOpType.add,
                        ), dve_chain, "dve-order")
                        nc.sync.dma_start(out.ap()[:, qs], t_sb[:])

    if split:
        _split_sync_waits(nc)
    return nc


def host_prep(inputs):
    """Full inputs -> list of 8 per-core input maps."""
    x = np.asarray(inputs["x"], np.float32)
    wq = np.asarray(inputs["wq"], np.float32)
    bq = np.asarray(inputs["bq"], np.float32)
    wk = np.asarray(inputs["wk"], np.float32)
    wv = np.asarray(inputs["wv"], np.float32)
    bv = np.asarray(inputs["bv"], np.float32)
    gamma = np.asarray(inputs["gamma"], np.float32)

    bf = ml_dtypes.bfloat16
    A = wq.T @ wk                     # (C, C):  A[c, i]
    u = wk.T @ bq                     # (C,)
    a_aug = np.concatenate([A, u[None, :]], axis=0).astype(bf)
    # gamma folded into v: out rows get gamma * v while the appended ones
    # column (softmax denominator) stays unscaled.
    gsc = float(gamma.reshape(-1)[0])
    wvT = (gsc * wv.T).astype(bf)
    wv_dup = np.concatenate([wvT, wvT], axis=0)
    bv8 = np.ascontiguousarray(
        np.tile((gsc * bv).astype(bf)[None, :], (1, 8))
    )

    xf = x.reshape(B, C, N)
    in_maps = []
    for core in range(N_CORES):
        b, h = core // 2, core % 2
        xq = xf[b][:, h * NQ : (h + 1) * NQ]
        xkb = np.ascontiguousarray(xf[b].astype(bf))
        in_maps.append(
            {
                "xk_bf": xkb,
                "xkd_bf": xkb,
                "xq_bf": np.ascontiguousarray(xq.astype(bf)),
                "xq_f32": np.ascontiguousarray(xq),
                "a_aug": a_aug,
                "wv_dup": wv_dup,
                "bv8": bv8,
            }
        )
    return in_maps


_NC_CACHE = None


def kernel(**inputs) -> np.ndarray:
    global _NC_CACHE
    from concourse.bass_utils import run_bass_kernel_spmd

    if _NC_CACHE is None:
        _NC_CACHE = build_nc()
    nc = _NC_CACHE
    in_maps = host_prep(inputs)
    res = run_bass_kernel_spmd(nc, in_maps, core_ids=list(range(N_CORES)))
    x = np.asarray(inputs["x"], np.float32)
    full = np.empty((B, C, N), np.float32)
    for core in range(N_CORES):
        b, h = core // 2, core % 2
        full[b][:, h * NQ : (h + 1) * NQ] = res.results[core]["out"]
    return full.reshape(x.shape)


if __name__ == "__main__":
    rng = np.random.default_rng(0)
    demo = {
        "x": rng.standard_normal((B, C, 16, 16, 16), dtype=np.float32),
        "wq": 0.05 * rng.standard_normal((8, C), dtype=np.float32),
        "bq": 0.05 * rng.standard_normal((8,), dtype=np.float32),
        "wk": 0.05 * rng.standard_normal((8, C), dtype=np.float32),
        "bk": 0.05 * rng.standard_normal((8,), dtype=np.float32),
        "wv": 0.05 * rng.standard_normal((C, C), dtype=np.float32),
        "bv": 0.05 * rng.standard_normal((C,), dtype=np.float32),
        "gamma": np.zeros((1,), np.float32),
    }
    print(kernel(**demo).shape)


# revision 28
# speedup vs baseline: 1.3331x; 1.1331x over previous
"""Trainium2 Bass kernel for nn_PamCell (spatial self-attention, B=4, C=64,
N=16^3=4096, CQ=8) on 8 NeuronCores.

Sharding: core i handles batch i//2 and query-half i%2 (2048 queries vs all
4096 keys). No collectives; host scatters inputs / gathers outputs.

Math: softmax rows are invariant to additive terms that depend only on the
query index, so with A = wq^T wk and u = wk^T bq,
    softmax(q k^T)[n, :] == softmax((A^T x_n + u) . x_m)[n, :]
which turns the QK contraction into a single 64-dim contraction against the
raw input as keys. Energies are in [-5, 5]: the softmax max-subtraction is
skipped (exp cannot overflow).

v2 changes vs the 119.5us baseline (which was ACT-serialized: the scalar
engine ran every exp tile, ~2.2us/granule, while the PE idled):
  - exp is split across engines per granule: ACT does the even key chunk
    (hardware Exp), DVE does the odd chunk with a magic-constant exp
    (bf16 bits of 2^t are linear in t: int16(e*184.665 + 16250.49) bitcast
    to bf16, +-3% sawtooth; RNE convert verified on HW).
  - ~4us of dummy matmuls at the head of the PE queue warm the HAM clock
    gate (2.4GHz) before the first real matmul instead of 15us in.
  - input DMAs spread over 4 engine queues; xq_f32 (epilogue-only) last.
  - bv is pre-loaded into the v^T PSUM accumulation by a K=1 ones matmul,
    dropping the DVE broadcast-add from the prologue.
  - granules run query-half-major so half 0's 1/rowsum (Ln+Exp on ACT, same
    table set) hides under half 1's main loop; the remaining tail is the
    4x [K=1 broadcast matmul -> ACT copy -> DVE mult/add -> DMA] pipeline.
"""

import sys

import numpy as np

try:
    import concourse.bass as bass
except ImportError:  # fresh interpreter without the env paths
    for _p in ("/root/.axon_site", "/root/.axon_site/_ro/trn_rl_repo",
               "/root/.axon_site/_ro/pypackages", "/opt/trn_rl_repo"):
        if _p not in sys.path:
            sys.path.append(_p)
    import concourse.bass as bass

import ml_dtypes

import concourse.tile as tile
from concourse import mybir
from concourse.vector_clock import ScopedClock

BF16 = mybir.dt.bfloat16
F32 = mybir.dt.float32
I16 = mybir.dt.int16
AF = mybir.ActivationFunctionType

B, C, N = 4, 64, 4096
NQ = N // 2          # queries per core
NKC = N // 128       # key chunks of 128
N_CORES = 8
NPAIR = NKC // 2     # 16 key-chunk pairs

S_MAGIC = 128.0 / float(np.log(2.0))   # 184.6650
B_MAGIC = 16256.0 - 5.5078             # RNE-rounded magic bias (HW-verified)
N_WARM = 14                            # dummy warm matmuls, 512 cols each


class _TileContextCompat(tile.TileContext):
    """Split the kernel-tail drain's sem waits across SP instructions;
    this walrus build allows only one sync-wait per CTRL instruction."""

    def _drain_and_barrier(self, tick_clock, wait_clock):
        probe = self.nc.sync.nop()
        wait_clock.add_sem_waits(
            probe.ins, ScopedClock({None: tick_clock.global_clock})
        )
        si = probe.ins.sync_info
        waits = list(si.on_wait) if si is not None else []
        if si is not None:
            probe.ins.sync_info = mybir.SyncInfo(
                on_wait=waits[:1], on_update=list(si.on_update)
            )
        for w in waits[1:]:
            nop = self.nc.sync.nop()
            nop.ins.sync_info = mybir.SyncInfo(on_wait=[w], on_update=[])

        self.nc.sync.drain()
        self.nc.all_engine_barrier()
        assert self.sems is not None
        popped = self.nc._tile_sem_poison_stack.pop()
        assert popped is self._sem_poison
        self.nc.clear_and_free_semaphores(list(self.sems.allocated().values()))
        self.nc.all_engine_barrier()


def _split_sync_waits(nc, max_waits=1):
    """This walrus build rejects instructions carrying more than one sync
    wait; hoist excess waits onto same-engine nops inserted just before."""
    for fn in nc.m.functions:
        for blk in fn.blocks:
            new = []
            changed = False
            for inst in blk.instructions:
                si = inst.sync_info
                if si is not None and si.on_wait and len(si.on_wait) > max_waits:
                    waits = list(si.on_wait)
                    excess = waits[:-max_waits]
                    for i in range(0, len(excess), max_waits):
                        nop = mybir.InstNoOp(
                            name=f"I-{nc.next_id()}-waitsplit", ins=[], outs=[]
                        )
                        nop.engine = inst.engine
                        nop.sync_info = mybir.SyncInfo(
                            on_wait=excess[i : i + max_waits], on_update=[]
                        )
                        new.append(nop)
                    inst.sync_info = mybir.SyncInfo(
                        on_wait=waits[-max_waits:], on_update=list(si.on_update)
                    )
                    changed = True
                new.append(inst)
            if changed:
                blk.instructions = new


def build_nc(split=True):
    nc = bass.Bass(
        "TRN2",
        target_bir_lowering=False,
        debug=False,
        enable_asserts=False,
    )
    xk_bf = nc.dram_tensor("xk_bf", (C, N), BF16, kind="ExternalInput")
    xkd_bf = nc.dram_tensor("xkd_bf", (C, N), BF16, kind="ExternalInput")
    xq_bf = nc.dram_tensor("xq_bf", (C, NQ), BF16, kind="ExternalInput")
    xq_f32 = nc.dram_tensor("xq_f32", (C, NQ), F32, kind="ExternalInput")
    a_aug = nc.dram_tensor("a_aug", (C + 1, C), BF16, kind="ExternalInput")
    wv_dup = nc.dram_tensor("wv_dup", (128, C), BF16, kind="ExternalInput")
    bv8 = nc.dram_tensor("bv8", (1, 512), BF16, kind="ExternalInput")
    scratch = nc.dram_tensor("scratch", (1, NQ), F32, kind="Internal")
    out = nc.dram_tensor("out", (C, NQ), F32, kind="ExternalOutput")

    with _TileContextCompat(nc) as tc:
        with tc.tile_pool(name="consts", bufs=1) as consts:
            # ---- persistent SBUF tensors ----
            xk2 = consts.tile([128, N], BF16, tag="xk2")     # keys, dup rows
            xq = consts.tile([C + 1, NQ], BF16, tag="xq")    # queries + ones
            xqf = consts.tile([C, NQ], F32, tag="xqf")
            a_sb = consts.tile([C + 1, C], BF16, tag="a_sb")
            wv_sb = consts.tile([128, C], BF16, tag="wv_sb")  # wv^T, dup rows
            bv_sb = consts.tile([1, 512], BF16, tag="bv_sb")  # gamma*bv, tiled 8x
            qb2 = consts.tile([128, NQ], BF16, tag="qb2")    # Q, dup rows
            vt = consts.tile([128, NKC, C + 1], BF16, tag="vt")
            ones_kb = consts.tile([1, 128], BF16, tag="ones_kb")
            warm_w = consts.tile([128, 512], BF16, tag="warm_w")
            r_sb = consts.tile([1, NQ], F32, tag="r_sb")
            rb_f = consts.tile([1, NQ], F32, tag="rb_f")
            rb_b = consts.tile([1, NQ], BF16, tag="rb_b")
            bc_sb = consts.tile([C, NQ], F32, tag="bc_sb")
            num_sb = consts.tile([C + 1, 1024], F32, tag="num_sb")
            warm_sb = consts.tile([1, 128], F32, tag="warm_sb")

            import bass_rust as _br

            pe_chain = [None]
            act_chain = [None]
            dve_chain = [None]

            def _chained(r, chain, reason="order"):
                if chain[0] is not None:
                    _br.add_dep_helper(r.ins, chain[0].ins, reason=reason)
                chain[0] = r
                return r

            # ---- memsets (vector first: warm matmuls depend on warm_w and
            # the gpsimd queue's preamble is slow) ----
            nc.vector.memset(warm_w[:], 1.0)
            nc.vector.memset(ones_kb[:], 1.0)
            nc.gpsimd.memset(xq[C : C + 1, :], 1.0)
            nc.gpsimd.memset(vt[:, :, C : C + 1], 1.0)

            # ---- input DMAs (gpsimd SWDGE DMAs are slow: sync/scalar only;
            # each quarter's dup follows its source so energies can start as
            # soon as the first quarter + dup land) ----
            nc.sync.dma_start(xq[:C, :], xq_bf.ap())
            nc.sync.dma_start(xk2[:C, bass.ts(0, N // 4)],
                              xk_bf.ap()[:, bass.ts(0, N // 4)])
            nc.sync.dma_start(xk2[C:, bass.ts(0, N // 4)],
                              xkd_bf.ap()[:, bass.ts(0, N // 4)])
            nc.sync.dma_start(xk2[:C, bass.ts(1, N // 4)],
                              xk_bf.ap()[:, bass.ts(1, N // 4)])
            nc.sync.dma_start(xk2[C:, bass.ts(1, N // 4)],
                              xkd_bf.ap()[:, bass.ts(1, N // 4)])
            nc.scalar.dma_start(a_sb[:], a_aug.ap())
            nc.scalar.dma_start(wv_sb[:], wv_dup.ap())
            nc.scalar.dma_start(bv_sb[:], bv8.ap())
            # the ~2.7us ACT table load waits until the small weights are
            # dispatched (it blocks the scalar queue while it runs)
            _chained(nc.scalar.activation(warm_sb[:], ones_kb[:], AF.Ln),
                     act_chain)
            nc.scalar.dma_start(xk2[:C, bass.ts(2, N // 4)],
                                xk_bf.ap()[:, bass.ts(2, N // 4)])
            nc.scalar.dma_start(xk2[C:, bass.ts(2, N // 4)],
                                xkd_bf.ap()[:, bass.ts(2, N // 4)])
            nc.scalar.dma_start(xk2[:C, bass.ts(3, N // 4)],
                                xk_bf.ap()[:, bass.ts(3, N // 4)])
            nc.scalar.dma_start(xk2[C:, bass.ts(3, N // 4)],
                                xkd_bf.ap()[:, bass.ts(3, N // 4)])
            # epilogue-only fp32 queries last
            nc.scalar.dma_start(xqf[:], xq_f32.ap())

            # ---- prologue ----
            # PSUM: q0(2) q1(2) vp(2) warm(1) = 7 banks; released before the
            # main loop so e0/e1/out_big can take all 8.
            with tc.tile_pool(name="psum_pro", bufs=1, space="PSUM") as pro:
                # dummy matmuls: keep the PE busy from t~0 so the HAM clock
                # gate flips to 2.4GHz before the first real matmul. Dense
                # pairs: alternate row groups, each writing its own PSUM
                # bank (concurrent row-group streams into one bank race).
                warm_ps = pro.tile([128, 1024], F32, tag="warm_ps")
                for i in range(N_WARM):
                    lo = 64 * (i % 2)
                    _chained(nc.tensor.matmul(
                        warm_ps[:, bass.ts(i % 2, 512)],
                        warm_w[lo : lo + 64, 0:128],
                        warm_w[lo : lo + 64, :],
                        start=True,
                        stop=True,
                        tile_position=(lo, 0),
                        skip_group_check=True,
                    ), pe_chain)

                # Q = a_aug^T xq_aug, written twice (col-tiled) so both
                # partition halves hold a copy for the row-tiled energy MMs.
                # Scaled by 128/ln2 on the way to SBUF so the DVE magic exp
                # is a single add; ACT exps undo it with their free scale.
                for j in range(NQ // 512):
                    qp = pro.tile([128, 512], F32, tag="qp", bufs=2,
                                  name="qp")
                    _chained(nc.tensor.matmul(
                        qp[:C, :],
                        a_sb[:],
                        xq[:, bass.ts(j, 512)],
                        start=True,
                        stop=True,
                        tile_position=(0, 0),
                    ), pe_chain)
                    _chained(nc.tensor.matmul(
                        qp[C:, :],
                        a_sb[:],
                        xq[:, bass.ts(j, 512)],
                        start=True,
                        stop=True,
                        tile_position=(0, 64),
                    ), pe_chain)
                    _chained(nc.vector.tensor_scalar_mul(
                        qb2[:, bass.ts(j, 512)], qp[:], S_MAGIC
                    ), dve_chain)

                # v^T per key chunk, row-tiled pairs; bv is pre-loaded into
                # the accumulation by a K=1 ones matmul so no bias add is
                # needed afterwards. NOT pe-chained: the scheduler slots
                # these into PE gaps while the first exps run.
                vt_r = vt.rearrange("p (t two) c -> p t two c", two=2)
                for g in range(2):
                    vp = pro.tile([128, 1024], F32, tag="vp", bufs=2, name="vp")
                    for half in range(2):
                        nc.tensor.matmul(
                            vp[:, bass.ts(half, 512)],
                            ones_kb[:, :],
                            bv_sb[:, :],
                            start=True,
                            stop=False,
                            skip_group_check=True,
                        )
                    for t in range(8):
                        pair = 8 * g + t
                        nc.tensor.matmul(
                            vp[:, bass.ts(t, C)],
                            xk2[:C, bass.ts(2 * pair, 128)],
                            wv_sb[:C, :],
                            start=False,
                            stop=True,
                            tile_position=(0, 0),
                            skip_group_check=True,
                        )
                        nc.tensor.matmul(
                            vp[:, bass.ds(512 + t * C, C)],
                            xk2[C:, bass.ts(2 * pair + 1, 128)],
                            wv_sb[C:, :],
                            start=False,
                            stop=True,
                            tile_position=(64, 0),
                            skip_group_check=True,
                        )
                    for half in range(2):
                        _chained(nc.scalar.copy(
                            vt_r[:, bass.ts(g, 8), half, :C],
                            vp[:, bass.ts(half, 512)].rearrange(
                                "p (t c) -> p t c", t=8
                            ),
                        ), act_chain)

            # ---- main loop (query-half-major) ----
            with (
                tc.tile_pool(name="psum_e", bufs=1, space="PSUM") as pe_pool,
                tc.tile_pool(name="psum_out", bufs=1, space="PSUM") as pout,
            ):
                out_big = pout.tile([C + 1, NQ], F32, tag="out_big",
                                    name="out_big")
                out_ps = [
                    out_big[:, bass.ts(qg, 512)] for qg in range(NQ // 512)
                ]
                with (
                    tc.tile_pool(name="pt_pool", bufs=6) as pt_pool,
                ):
                    # granule = (pair, qh), qh-major: granules 0-15 cover
                    # query half 0 over all key pairs, 16-31 half 1. Half 0's
                    # epilogue recip overlaps half 1's compute.
                    NG = NKC  # 32 granules
                    gr = [(pair, qh) for qh in range(2) for pair in range(NPAIR)]

                    def energies(g):
                        """All 4 energy MMs of granule g, one single-bank
                        [128,512] PSUM tile per (half, j): interleaved h0/h64
                        so adjacent MMs run concurrently in disjoint row
                        groups, and each tile is released to the exp engines
                        after a single MM so the energy->exp->bank-free
                        recurrence stays off the critical path."""
                        pair, qh = gr[g]
                        qoff = qh * 1024
                        es = [[None, None], [None, None]]
                        for j in range(2):
                            for half in range(2):
                                es[half][j] = pe_pool.tile(
                                    [128, 512], F32, tag=f"e{half}{j}",
                                    name=f"e{half}{j}"
                                )
                                mc = 2 * pair + half
                                lo = C * half
                                _chained(
                                    nc.tensor.matmul(
                                        es[half][j][:],
                                        xk2[lo : lo + C, bass.ts(mc, 128)],
                                        qb2[lo : lo + C,
                                            bass.ds(qoff + j * 512, 512)],
                                        start=True,
                                        stop=True,
                                        tile_position=(lo, 0),
                                    ),
                                    pe_chain,
                                    "pe-order",
                                )
                        return es

                    def do_exp_act(e2):
                        """Even chunk: hardware Exp on the scalar engine.
                        Energies carry the 128/ln2 magic scale; undo it with
                        the instruction's free affine scale."""
                        pt = pt_pool.tile([128, 1024], BF16, tag="pt0",
                                          name="pt0")
                        for j in range(2):
                            _chained(
                                nc.scalar.activation(
                                    pt[:, bass.ts(j, 512)], e2[j][:], AF.Exp,
                                    scale=1.0 / S_MAGIC),
                                act_chain,
                                "act-order",
                            )
                        return pt

                    def do_exp_dve(e2):
                        """Odd chunk: magic-constant exp on the vector
                        engine: bf16 bits of 2^(e/ln2) are int16(e_s + b)
                        with e_s pre-scaled in qb2."""
                        pt = pt_pool.tile([128, 1024], BF16, tag="pt1",
                                          name="pt1")
                        for j in range(2):
                            _chained(
                                nc.vector.tensor_scalar_add(
                                    pt[:, bass.ts(j, 512)].bitcast(I16),
                                    e2[j][:], B_MAGIC,
                                ),
                                dve_chain,
                                "dve-order",
                            )
                        return pt

                    def outs(g, half, pt):
                        pair, qh = gr[g]
                        mc = 2 * pair + half
                        for j in range(2):
                            qg = 2 * qh + j
                            _chained(
                                nc.tensor.matmul(
                                    out_ps[qg][:],
                                    vt[:, mc, :],
                                    pt[:, bass.ts(j, 512)],
                                    start=(pair == 0),
                                    stop=(pair == NPAIR - 1),
                                    skip_group_check=True,
                                ),
                                pe_chain,
                                "pe-order",
                            )

                    def bcast(qg):
                        """Broadcast 1/rowsum across 64 partitions by a DRAM
                        round trip: row out, stride-0 read back. Both DMAs on
                        the sync queue, so they execute in order."""
                        qs = bass.ts(qg, 512)
                        nc.sync.dma_start(scratch.ap()[:, qs], rb_f[:, qs])
                        nc.sync.dma_start(
                            bc_sb[:, qs],
                            scratch.ap()[:, qs].broadcast_to((C, 512)),
                        )

                    # half-0 epilogue, one step per half-1 granule. out_big
                    # subtile reads mid-loop resolve against the whole-tile
                    # write chain (they'd wait for the CURRENT granule's
                    # accumulation and dam up the ACT queue), so stage
                    # half 0's numerator+rowsum to SBUF with ONE copy, then
                    # recip / divide / add all read SBUF: Ln+Exp chunks on
                    # ACT, divide+residual on the otherwise-idle gpsimd.
                    def epi0_step(k):
                        if k == 0:
                            _chained(nc.scalar.copy(
                                num_sb[:, :], out_big[:, 0:1024],
                            ), act_chain, "act-order")
                            return
                        if k in (1, 2, 3, 4):
                            if k % 2 == 1:
                                _chained(nc.scalar.activation(
                                    r_sb[:, bass.ts(k // 2, 512)],
                                    num_sb[C : C + 1, bass.ts(k // 2, 512)],
                                    AF.Ln,
                                ), act_chain, "act-order")
                            else:
                                _chained(nc.scalar.activation(
                                    rb_f[:, bass.ts(k // 2 - 1, 512)],
                                    r_sb[:, bass.ts(k // 2 - 1, 512)],
                                    AF.Exp, scale=-1.0,
                                ), act_chain, "act-order")
                            return
                        if k == 5:
                            bcast(0)
                            bcast(1)
                            return
                        if k in (7, 9):
                            qg = (k - 7) // 2
                            qs = bass.ts(qg, 512)
                            nc.gpsimd.tensor_tensor(
                                num_sb[:C, qs], num_sb[:C, qs], bc_sb[:, qs],
                                mybir.AluOpType.mult,
                            )
                            return
                        if k in (8, 10):
                            qg = (k - 8) // 2
                            qs = bass.ts(qg, 512)
                            nc.gpsimd.tensor_tensor(
                                num_sb[:C, qs], num_sb[:C, qs], xqf[:, qs],
                                mybir.AluOpType.add,
                            )
                            nc.sync.dma_start(out.ap()[:, qs],
                                              num_sb[:C, qs])
                            return

                    es = {0: energies(0)}
                    for g in range(NG):
                        eA2, eB2 = es.pop(g)
                        ptA = do_exp_act(eA2)
                        ptB = do_exp_dve(eB2)
                        if g + 1 < NG:
                            es[g + 1] = energies(g + 1)
                        outs(g, 0, ptA)
                        outs(g, 1, ptB)
                        if g >= NPAIR:
                            epi0_step(g - NPAIR)

                # ---- tail: half-1 epilogue, pipelined per query group.
                # The DRAM round-trip broadcast costs ~4.7us serial, so here
                # (nothing left to hide under) broadcast via K=1 ones matmul
                # into the freed energy banks + ACT copy instead; the recip
                # Exp writes bf16 directly for the matmul rhs. ----
                with tc.tile_pool(name="epi", bufs=2) as epi:
                    for qg in range(2, 4):
                        qs = bass.ts(qg, 512)
                        _chained(nc.scalar.activation(
                            r_sb[:, qs], out_big[C : C + 1, qs], AF.Ln
                        ), act_chain, "act-order")
                        _chained(nc.scalar.activation(
                            rb_b[:, qs], r_sb[:, qs], AF.Exp, scale=-1.0
                        ), act_chain, "act-order")
                        bc_ps = pe_pool.tile(
                            [C, 512], F32, tag=f"e{qg % 2}0",
                            name=f"bc{qg}"
                        )
                        _chained(nc.tensor.matmul(
                            bc_ps[:],
                            ones_kb[:, :C],
                            rb_b[:, qs],
                            start=True,
                            stop=True,
                        ), pe_chain, "pe-order")
                        _chained(nc.scalar.copy(
                            bc_sb[:, qs], bc_ps[:]
                        ), act_chain, "act-order")
                        t_sb = epi.tile([C, 512], F32, tag="t_sb")
                        _chained(nc.vector.tensor_tensor(
                            t_sb[:], out_ps[qg][:C, :], bc_sb[:, qs],
                            mybir.AluOpType.mult,
                        ), dve_chain, "dve-order")
                        _chained(nc.vector.tensor_tensor(
                            t_sb[:], t_sb[:], xqf[:, qs],
                            mybir.AluOpType.add,
                        ), dve_chain, "dve-order")
                        nc.sync.dma_start(out.ap()[:, qs], t_sb[:])

    if split:
        _split_sync_waits(nc)
    return nc


def host_prep(inputs):
    """Full inputs -> list of 8 per-core input maps."""
    x = np.asarray(inputs["x"], np.float32)
    wq = np.asarray(inputs["wq"], np.float32)
    bq = np.asarray(inputs["bq"], np.float32)
    wk = np.asarray(inputs["wk"], np.float32)
    wv = np.asarray(inputs["wv"], np.float32)
    bv = np.asarray(inputs["bv"], np.float32)
    gamma = np.asarray(inputs["gamma"], np.float32)

    bf = ml_dtypes.bfloat16
    A = wq.T @ wk                     # (C, C):  A[c, i]
    u = wk.T @ bq                     # (C,)
    a_aug = np.concatenate([A, u[None, :]], axis=0).astype(bf)
    # gamma folded into v: out rows get gamma * v while the appended ones
    # column (softmax denominator) stays unscaled.
    gsc = float(gamma.reshape(-1)[0])
    wvT = (gsc * wv.T).astype(bf)
    wv_dup = np.concatenate([wvT, wvT], axis=0)
    bv8 = np.ascontiguousarray(
        np.tile((gsc * bv).astype(bf)[None, :], (1, 8))
    )

    xf = x.reshape(B, C, N)
    in_maps = []
    for core in range(N_CORES):
        b, h = core // 2, core % 2
        xq = xf[b][:, h * NQ : (h + 1) * NQ]
        xkb = np.ascontiguousarray(xf[b].astype(bf))
        in_maps.append(
            {
                "xk_bf": xkb,
                "xkd_bf": xkb,
                "xq_bf": np.ascontiguousarray(xq.astype(bf)),
                "xq_f32": np.ascontiguousarray(xq),
                "a_aug": a_aug,
                "wv_dup": wv_dup,
                "bv8": bv8,
            }
        )
    return in_maps


_NC_CACHE = None


def kernel(**inputs) -> np.ndarray:
    global _NC_CACHE
    from concourse.bass_utils import run_bass_kernel_spmd

    if _NC_CACHE is None:
        _NC_CACHE = build_nc()
    nc = _NC_CACHE
    in_maps = host_prep(inputs)
    res = run_bass_kernel_spmd(nc, in_maps, core_ids=list(range(N_CORES)))
    x = np.asarray(inputs["x"], np.float32)
    full = np.empty((B, C, N), np.float32)
    for core in range(N_CORES):
        b, h = core // 2, core % 2
        full[b][:, h * NQ : (h + 1) * NQ] = res.results[core]["out"]
    return full.reshape(x.shape)


if __name__ == "__main__":
    rng = np.random.default_rng(0)
    demo = {
        "x": rng.standard_normal((B, C, 16, 16, 16), dtype=np.float32),
        "wq": 0.05 * rng.standard_normal((8, C), dtype=np.float32),
        "bq": 0.05 * rng.standard_normal((8,), dtype=np.float32),
        "wk": 0.05 * rng.standard_normal((8, C), dtype=np.float32),
        "bk": 0.05 * rng.standard_normal((8,), dtype=np.float32),
        "wv": 0.05 * rng.standard_normal((C, C), dtype=np.float32),
        "bv": 0.05 * rng.standard_normal((C,), dtype=np.float32),
        "gamma": np.zeros((1,), np.float32),
    }
    print(kernel(**demo).shape)


# revision 31
# speedup vs baseline: 1.3426x; 1.0071x over previous
"""Trainium2 Bass kernel for nn_PamCell (spatial self-attention, B=4, C=64,
N=16^3=4096, CQ=8) on 8 NeuronCores.

Sharding: core i handles batch i//2 and query-half i%2 (2048 queries vs all
4096 keys). No collectives; host scatters inputs / gathers outputs.

Math: softmax rows are invariant to additive terms that depend only on the
query index, so with A = wq^T wk and u = wk^T bq,
    softmax(q k^T)[n, :] == softmax((A^T x_n + u) . x_m)[n, :]
which turns the QK contraction into a single 64-dim contraction against the
raw input as keys. Energies are in [-5, 5]: the softmax max-subtraction is
skipped (exp cannot overflow).

v2 changes vs the 119.5us baseline (which was ACT-serialized: the scalar
engine ran every exp tile, ~2.2us/granule, while the PE idled):
  - exp is split across engines per granule: ACT does the even key chunk
    (hardware Exp), DVE does the odd chunk with a magic-constant exp
    (bf16 bits of 2^t are linear in t: int16(e*184.665 + 16250.49) bitcast
    to bf16, +-3% sawtooth; RNE convert verified on HW).
  - ~4us of dummy matmuls at the head of the PE queue warm the HAM clock
    gate (2.4GHz) before the first real matmul instead of 15us in.
  - input DMAs spread over 4 engine queues; xq_f32 (epilogue-only) last.
  - bv is pre-loaded into the v^T PSUM accumulation by a K=1 ones matmul,
    dropping the DVE broadcast-add from the prologue.
  - granules run query-half-major so half 0's 1/rowsum (Ln+Exp on ACT, same
    table set) hides under half 1's main loop; the remaining tail is the
    4x [K=1 broadcast matmul -> ACT copy -> DVE mult/add -> DMA] pipeline.
"""

import sys

import numpy as np

try:
    import concourse.bass as bass
except ImportError:  # fresh interpreter without the env paths
    for _p in ("/root/.axon_site", "/root/.axon_site/_ro/trn_rl_repo",
               "/root/.axon_site/_ro/pypackages", "/opt/trn_rl_repo"):
        if _p not in sys.path:
            sys.path.append(_p)
    import concourse.bass as bass

import ml_dtypes

import concourse.tile as tile
from concourse import mybir
from concourse.vector_clock import ScopedClock

BF16 = mybir.dt.bfloat16
F32 = mybir.dt.float32
I16 = mybir.dt.int16
AF = mybir.ActivationFunctionType

B, C, N = 4, 64, 4096
NQ = N // 2          # queries per core
NKC = N // 128       # key chunks of 128
N_CORES = 8
NPAIR = NKC // 2     # 16 key-chunk pairs

S_MAGIC = 128.0 / float(np.log(2.0))   # 184.6650
B_MAGIC = 16256.0 - 5.5078             # RNE-rounded magic bias (HW-verified)
N_WARM = 14                            # dummy warm matmuls, 512 cols each


class _TileContextCompat(tile.TileContext):
    """Split the kernel-tail drain's sem waits across SP instructions;
    this walrus build allows only one sync-wait per CTRL instruction."""

    def _drain_and_barrier(self, tick_clock, wait_clock):
        probe = self.nc.sync.nop()
        wait_clock.add_sem_waits(
            probe.ins, ScopedClock({None: tick_clock.global_clock})
        )
        si = probe.ins.sync_info
        waits = list(si.on_wait) if si is not None else []
        if si is not None:
            probe.ins.sync_info = mybir.SyncInfo(
                on_wait=waits[:1], on_update=list(si.on_update)
            )
        for w in waits[1:]:
            nop = self.nc.sync.nop()
            nop.ins.sync_info = mybir.SyncInfo(on_wait=[w], on_update=[])

        self.nc.sync.drain()
        self.nc.all_engine_barrier()
        assert self.sems is not None
        popped = self.nc._tile_sem_poison_stack.pop()
        assert popped is self._sem_poison
        self.nc.clear_and_free_semaphores(list(self.sems.allocated().values()))
        self.nc.all_engine_barrier()


def _split_sync_waits(nc, max_waits=1):
    """This walrus build rejects instructions carrying more than one sync
    wait; hoist excess waits onto same-engine nops inserted just before."""
    for fn in nc.m.functions:
        for blk in fn.blocks:
            new = []
            changed = False
            for inst in blk.instructions:
                si = inst.sync_info
                if si is not None and si.on_wait and len(si.on_wait) > max_waits:
                    waits = list(si.on_wait)
                    excess = waits[:-max_waits]
                    for i in range(0, len(excess), max_waits):
                        nop = mybir.InstNoOp(
                            name=f"I-{nc.next_id()}-waitsplit", ins=[], outs=[]
                        )
                        nop.engine = inst.engine
                        nop.sync_info = mybir.SyncInfo(
                            on_wait=excess[i : i + max_waits], on_update=[]
                        )
                        new.append(nop)
                    inst.sync_info = mybir.SyncInfo(
                        on_wait=waits[-max_waits:], on_update=list(si.on_update)
                    )
                    changed = True
                new.append(inst)
            if changed:
                blk.instructions = new


def build_nc(split=True):
    nc = bass.Bass(
        "TRN2",
        target_bir_lowering=False,
        debug=False,
        enable_asserts=False,
    )
    xk_bf = nc.dram_tensor("xk_bf", (C, N), BF16, kind="ExternalInput")
    xkd_bf = nc.dram_tensor("xkd_bf", (C, N), BF16, kind="ExternalInput")
    xq_bf = nc.dram_tensor("xq_bf", (C, NQ), BF16, kind="ExternalInput")
    xq_f32 = nc.dram_tensor("xq_f32", (C, NQ), F32, kind="ExternalInput")
    a_aug = nc.dram_tensor("a_aug", (C + 1, C), BF16, kind="ExternalInput")
    wv_dup = nc.dram_tensor("wv_dup", (128, C), BF16, kind="ExternalInput")
    bv8 = nc.dram_tensor("bv8", (1, 512), BF16, kind="ExternalInput")
    scratch = nc.dram_tensor("scratch", (1, NQ), F32, kind="Internal")
    out = nc.dram_tensor("out", (C, NQ), F32, kind="ExternalOutput")

    with _TileContextCompat(nc) as tc:
        with tc.tile_pool(name="consts", bufs=1) as consts:
            # ---- persistent SBUF tensors ----
            xk2 = consts.tile([128, N], BF16, tag="xk2")     # keys, dup rows
            xq = consts.tile([C + 1, NQ], BF16, tag="xq")    # queries + ones
            xqf = consts.tile([C, NQ], F32, tag="xqf")
            a_sb = consts.tile([C + 1, C], BF16, tag="a_sb")
            wv_sb = consts.tile([128, C], BF16, tag="wv_sb")  # wv^T, dup rows
            bv_sb = consts.tile([1, 512], BF16, tag="bv_sb")  # gamma*bv, tiled 8x
            qb2 = consts.tile([128, NQ], BF16, tag="qb2")    # Q, dup rows
            vt = consts.tile([128, NKC, C + 1], BF16, tag="vt")
            ones_kb = consts.tile([1, 128], BF16, tag="ones_kb")
            warm_w = consts.tile([128, 512], BF16, tag="warm_w")
            r_sb = consts.tile([1, NQ], F32, tag="r_sb")
            rb_f = consts.tile([1, NQ], F32, tag="rb_f")
            rb_b = consts.tile([1, NQ], BF16, tag="rb_b")
            bc_sb = consts.tile([C, NQ], F32, tag="bc_sb")
            num_sb = consts.tile([C + 1, 1024], F32, tag="num_sb")
            warm_sb = consts.tile([1, 128], F32, tag="warm_sb")

            import bass_rust as _br

            pe_chain = [None]
            act_chain = [None]
            dve_chain = [None]

            def _chained(r, chain, reason="order"):
                if chain[0] is not None:
                    _br.add_dep_helper(r.ins, chain[0].ins, reason=reason)
                chain[0] = r
                return r

            # ---- memsets (vector first: warm matmuls depend on warm_w and
            # the gpsimd queue's preamble is slow) ----
            nc.vector.memset(warm_w[:], 1.0)
            nc.vector.memset(ones_kb[:], 1.0)
            nc.gpsimd.memset(xq[C : C + 1, :], 1.0)
            nc.gpsimd.memset(vt[:, :, C : C + 1], 1.0)

            # ---- input DMAs (gpsimd SWDGE DMAs are slow: sync/scalar only;
            # each quarter's dup follows its source so energies can start as
            # soon as the first quarter + dup land) ----
            # tiny weights first on sync (the scalar queue stalls ~3us on
            # the ACT table load and the prologue v^T matmuls need wv/bv)
            nc.sync.dma_start(bv_sb[:], bv8.ap())
            nc.sync.dma_start(wv_sb[:], wv_dup.ap())
            nc.sync.dma_start(xq[:C, :], xq_bf.ap())
            nc.sync.dma_start(xk2[:C, bass.ts(0, N // 4)],
                              xk_bf.ap()[:, bass.ts(0, N // 4)])
            nc.sync.dma_start(xk2[C:, bass.ts(0, N // 4)],
                              xkd_bf.ap()[:, bass.ts(0, N // 4)])
            nc.sync.dma_start(xk2[:C, bass.ts(1, N // 4)],
                              xk_bf.ap()[:, bass.ts(1, N // 4)])
            nc.sync.dma_start(xk2[C:, bass.ts(1, N // 4)],
                              xkd_bf.ap()[:, bass.ts(1, N // 4)])
            nc.scalar.dma_start(a_sb[:], a_aug.ap())
            # the ~2.7us ACT table load runs here, after the a_aug dispatch
            _chained(nc.scalar.activation(warm_sb[:], ones_kb[:], AF.Ln),
                     act_chain)
            nc.scalar.dma_start(xk2[:C, bass.ts(2, N // 4)],
                                xk_bf.ap()[:, bass.ts(2, N // 4)])
            nc.scalar.dma_start(xk2[C:, bass.ts(2, N // 4)],
                                xkd_bf.ap()[:, bass.ts(2, N // 4)])
            nc.scalar.dma_start(xk2[:C, bass.ts(3, N // 4)],
                                xk_bf.ap()[:, bass.ts(3, N // 4)])
            nc.scalar.dma_start(xk2[C:, bass.ts(3, N // 4)],
                                xkd_bf.ap()[:, bass.ts(3, N // 4)])
            # epilogue-only fp32 queries last
            nc.scalar.dma_start(xqf[:], xq_f32.ap())

            # ---- prologue ----
            # PSUM: q0(2) q1(2) vp(2) warm(1) = 7 banks; released before the
            # main loop so e0/e1/out_big can take all 8.
            with tc.tile_pool(name="psum_pro", bufs=1, space="PSUM") as pro:
                # dummy matmuls: keep the PE busy from t~0 so the HAM clock
                # gate flips to 2.4GHz before the first real matmul. Dense
                # pairs: alternate row groups, each writing its own PSUM
                # bank (concurrent row-group streams into one bank race).
                warm_ps = pro.tile([128, 1024], F32, tag="warm_ps")
                for i in range(N_WARM):
                    lo = 64 * (i % 2)
                    _chained(nc.tensor.matmul(
                        warm_ps[:, bass.ts(i % 2, 512)],
                        warm_w[lo : lo + 64, 0:128],
                        warm_w[lo : lo + 64, :],
                        start=True,
                        stop=True,
                        tile_position=(lo, 0),
                        skip_group_check=True,
                    ), pe_chain)

                # Q = a_aug^T xq_aug, written twice (col-tiled) so both
                # partition halves hold a copy for the row-tiled energy MMs.
                # Scaled by 128/ln2 on the way to SBUF so the DVE magic exp
                # is a single add; ACT exps undo it with their free scale.
                for j in range(NQ // 512):
                    qp = pro.tile([128, 512], F32, tag="qp", bufs=2,
                                  name="qp")
                    _chained(nc.tensor.matmul(
                        qp[:C, :],
                        a_sb[:],
                        xq[:, bass.ts(j, 512)],
                        start=True,
                        stop=True,
                        tile_position=(0, 0),
                    ), pe_chain)
                    _chained(nc.tensor.matmul(
                        qp[C:, :],
                        a_sb[:],
                        xq[:, bass.ts(j, 512)],
                        start=True,
                        stop=True,
                        tile_position=(0, 64),
                    ), pe_chain)
                    _chained(nc.vector.tensor_scalar_mul(
                        qb2[:, bass.ts(j, 512)], qp[:], S_MAGIC
                    ), dve_chain)

                # v^T per key chunk, row-tiled pairs; bv is pre-loaded into
                # the accumulation by a K=1 ones matmul so no bias add is
                # needed afterwards. NOT pe-chained: the scheduler slots
                # these into PE gaps while the first exps run.
                vt_r = vt.rearrange("p (t two) c -> p t two c", two=2)
                for g in range(2):
                    vp = pro.tile([128, 1024], F32, tag="vp", bufs=2, name="vp")
                    for half in range(2):
                        nc.tensor.matmul(
                            vp[:, bass.ts(half, 512)],
                            ones_kb[:, :],
                            bv_sb[:, :],
                            start=True,
                            stop=False,
                            skip_group_check=True,
                        )
                    for t in range(8):
                        pair = 8 * g + t
                        nc.tensor.matmul(
                            vp[:, bass.ts(t, C)],
                            xk2[:C, bass.ts(2 * pair, 128)],
                            wv_sb[:C, :],
                            start=False,
                            stop=True,
                            tile_position=(0, 0),
                            skip_group_check=True,
                        )
                        nc.tensor.matmul(
                            vp[:, bass.ds(512 + t * C, C)],
                            xk2[C:, bass.ts(2 * pair + 1, 128)],
                            wv_sb[C:, :],
                            start=False,
                            stop=True,
                            tile_position=(64, 0),
                            skip_group_check=True,
                        )
                    for half in range(2):
                        _chained(nc.scalar.copy(
                            vt_r[:, bass.ts(g, 8), half, :C],
                            vp[:, bass.ts(half, 512)].rearrange(
                                "p (t c) -> p t c", t=8
                            ),
                        ), act_chain)

            # ---- main loop (query-half-major) ----
            with (
                tc.tile_pool(name="psum_e", bufs=1, space="PSUM") as pe_pool,
                tc.tile_pool(name="psum_out", bufs=1, space="PSUM") as pout,
            ):
                out_big = pout.tile([C + 1, NQ], F32, tag="out_big",
                                    name="out_big")
                out_ps = [
                    out_big[:, bass.ts(qg, 512)] for qg in range(NQ // 512)
                ]
                with (
                    tc.tile_pool(name="pt_pool", bufs=4) as pt_pool,
                ):
                    # granule = (pair, qh), qh-major: granules 0-15 cover
                    # query half 0 over all key pairs, 16-31 half 1. Half 0's
                    # epilogue recip overlaps half 1's compute.
                    NG = NKC  # 32 granules
                    gr = [(pair, qh) for qh in range(2) for pair in range(NPAIR)]

                    def energies(g):
                        """All 4 energy MMs of granule g, one single-bank
                        [128,512] PSUM tile per (half, j): interleaved h0/h64
                        so adjacent MMs run concurrently in disjoint row
                        groups, and each tile is released to the exp engines
                        after a single MM so the energy->exp->bank-free
                        recurrence stays off the critical path."""
                        pair, qh = gr[g]
                        qoff = qh * 1024
                        es = [[None, None], [None, None]]
                        for j in range(2):
                            for half in range(2):
                                es[half][j] = pe_pool.tile(
                                    [128, 512], F32, tag=f"e{half}{j}",
                                    name=f"e{half}{j}"
                                )
                                mc = 2 * pair + half
                                lo = C * half
                                _chained(
                                    nc.tensor.matmul(
                                        es[half][j][:],
                                        xk2[lo : lo + C, bass.ts(mc, 128)],
                                        qb2[lo : lo + C,
                                            bass.ds(qoff + j * 512, 512)],
                                        start=True,
                                        stop=True,
                                        tile_position=(lo, 0),
                                    ),
                                    pe_chain,
                                    "pe-order",
                                )
                        return es

                    def do_exp_act(e2):
                        """Even chunk: hardware Exp on the scalar engine.
                        Energies carry the 128/ln2 magic scale; undo it with
                        the instruction's free affine scale."""
                        pt = pt_pool.tile([128, 1024], BF16, tag="pt0",
                                          name="pt0")
                        for j in range(2):
                            _chained(
                                nc.scalar.activation(
                                    pt[:, bass.ts(j, 512)], e2[j][:], AF.Exp,
                                    scale=1.0 / S_MAGIC),
                                act_chain,
                                "act-order",
                            )
                        return pt

                    def do_exp_dve(e2):
                        """Odd chunk: magic-constant exp on the vector
                        engine: bf16 bits of 2^(e/ln2) are int16(e_s + b)
                        with e_s pre-scaled in qb2."""
                        pt = pt_pool.tile([128, 1024], BF16, tag="pt1",
                                          name="pt1")
                        for j in range(2):
                            _chained(
                                nc.vector.tensor_scalar_add(
                                    pt[:, bass.ts(j, 512)].bitcast(I16),
                                    e2[j][:], B_MAGIC,
                                ),
                                dve_chain,
                                "dve-order",
                            )
                        return pt

                    def outs(g, half, pt):
                        pair, qh = gr[g]
                        mc = 2 * pair + half
                        for j in range(2):
                            qg = 2 * qh + j
                            _chained(
                                nc.tensor.matmul(
                                    out_ps[qg][:],
                                    vt[:, mc, :],
                                    pt[:, bass.ts(j, 512)],
                                    start=(pair == 0),
                                    stop=(pair == NPAIR - 1),
                                    skip_group_check=True,
                                ),
                                pe_chain,
                                "pe-order",
                            )

                    def bcast(qg):
                        """Broadcast 1/rowsum across 64 partitions by a DRAM
                        round trip: row out, stride-0 read back. Both DMAs on
                        the sync queue, so they execute in order."""
                        qs = bass.ts(qg, 512)
                        nc.sync.dma_start(scratch.ap()[:, qs], rb_f[:, qs])
                        nc.sync.dma_start(
                            bc_sb[:, qs],
                            scratch.ap()[:, qs].broadcast_to((C, 512)),
                        )

                    # half-0 epilogue, one step per half-1 granule. out_big
                    # subtile reads mid-loop resolve against the whole-tile
                    # write chain (they'd wait for the CURRENT granule's
                    # accumulation and dam up the ACT queue), so stage
                    # half 0's numerator+rowsum to SBUF with ONE copy, then
                    # recip / divide / add all read SBUF: Ln+Exp chunks on
                    # ACT, divide+residual on the otherwise-idle gpsimd.
                    def epi0_step(k):
                        if k == 0:
                            _chained(nc.scalar.copy(
                                num_sb[:, :], out_big[:, 0:1024],
                            ), act_chain, "act-order")
                            return
                        if k in (1, 2, 3, 4):
                            if k % 2 == 1:
                                _chained(nc.scalar.activation(
                                    r_sb[:, bass.ts(k // 2, 512)],
                                    num_sb[C : C + 1, bass.ts(k // 2, 512)],
                                    AF.Ln,
                                ), act_chain, "act-order")
                            else:
                                _chained(nc.scalar.activation(
                                    rb_f[:, bass.ts(k // 2 - 1, 512)],
                                    r_sb[:, bass.ts(k // 2 - 1, 512)],
                                    AF.Exp, scale=-1.0,
                                ), act_chain, "act-order")
                            return
                        if k == 5:
                            bcast(0)
                            bcast(1)
                            return
                        if k in (7, 9):
                            qg = (k - 7) // 2
                            qs = bass.ts(qg, 512)
                            nc.gpsimd.tensor_tensor(
                                num_sb[:C, qs], num_sb[:C, qs], bc_sb[:, qs],
                                mybir.AluOpType.mult,
                            )
                            return
                        if k in (8, 10):
                            qg = (k - 8) // 2
                            qs = bass.ts(qg, 512)
                            nc.gpsimd.tensor_tensor(
                                num_sb[:C, qs], num_sb[:C, qs], xqf[:, qs],
                                mybir.AluOpType.add,
                            )
                            nc.sync.dma_start(out.ap()[:, qs],
                                              num_sb[:C, qs])
                            return

                    es = {0: energies(0)}
                    for g in range(NG):
                        eA2, eB2 = es.pop(g)
                        ptA = do_exp_act(eA2)
                        ptB = do_exp_dve(eB2)
                        if g + 1 < NG:
                            es[g + 1] = energies(g + 1)
                        outs(g, 0, ptA)
                        outs(g, 1, ptB)
                        if g >= NPAIR:
                            epi0_step(g - NPAIR)

                # ---- tail: half-1 epilogue, pipelined per query group.
                # The DRAM round-trip broadcast costs ~4.7us serial, so here
                # (nothing left to hide under) broadcast via K=1 ones matmul
                # into the freed energy banks + ACT copy instead; the recip
                # Exp writes bf16 directly for the matmul rhs. ----
                with tc.tile_pool(name="epi", bufs=2) as epi:
                    hs = bass.ds(1024, 1024)
                    _chained(nc.scalar.activation(
                        r_sb[:, hs], out_big[C : C + 1, hs], AF.Ln
                    ), act_chain, "act-order")
                    _chained(nc.scalar.activation(
                        rb_b[:, hs], r_sb[:, hs], AF.Exp, scale=-1.0
                    ), act_chain, "act-order")
                    for qg in range(2, 4):
                        qs = bass.ts(qg, 512)
                        bc_ps = pe_pool.tile(
                            [C, 512], F32, tag=f"e{qg % 2}0",
                            name=f"bc{qg}"
                        )
                        _chained(nc.tensor.matmul(
                            bc_ps[:],
                            ones_kb[:, :C],
                            rb_b[:, qs],
                            start=True,
                            stop=True,
                        ), pe_chain, "pe-order")
                        _chained(nc.scalar.copy(
                            bc_sb[:, qs], bc_ps[:]
                        ), act_chain, "act-order")
                        t_sb = epi.tile([C, 512], F32, tag="t_sb")
                        _chained(nc.vector.tensor_tensor(
                            t_sb[:], out_ps[qg][:C, :], bc_sb[:, qs],
                            mybir.AluOpType.mult,
                        ), dve_chain, "dve-order")
                        _chained(nc.vector.tensor_tensor(
                            t_sb[:], t_sb[:], xqf[:, qs],
                            mybir.AluOpType.add,
                        ), dve_chain, "dve-order")
                        nc.sync.dma_start(out.ap()[:, qs], t_sb[:])

    if split:
        _split_sync_waits(nc)
    return nc


def host_prep(inputs):
    """Full inputs -> list of 8 per-core input maps."""
    x = np.asarray(inputs["x"], np.float32)
    wq = np.asarray(inputs["wq"], np.float32)
    bq = np.asarray(inputs["bq"], np.float32)
    wk = np.asarray(inputs["wk"], np.float32)
    wv = np.asarray(inputs["wv"], np.float32)
    bv = np.asarray(inputs["bv"], np.float32)
    gamma = np.asarray(inputs["gamma"], np.float32)

    bf = ml_dtypes.bfloat16
    A = wq.T @ wk                     # (C, C):  A[c, i]
    u = wk.T @ bq                     # (C,)
    a_aug = np.concatenate([A, u[None, :]], axis=0).astype(bf)
    # gamma folded into v: out rows get gamma * v while the appended ones
    # column (softmax denominator) stays unscaled.
    gsc = float(gamma.reshape(-1)[0])
    wvT = (gsc * wv.T).astype(bf)
    wv_dup = np.concatenate([wvT, wvT], axis=0)
    bv8 = np.ascontiguousarray(
        np.tile((gsc * bv).astype(bf)[None, :], (1, 8))
    )

    xf = x.reshape(B, C, N)
    in_maps = []
    for core in range(N_CORES):
        b, h = core // 2, core % 2
        xq = xf[b][:, h * NQ : (h + 1) * NQ]
        xkb = np.ascontiguousarray(xf[b].astype(bf))
        in_maps.append(
            {
                "xk_bf": xkb,
                "xkd_bf": xkb,
                "xq_bf": np.ascontiguousarray(xq.astype(bf)),
                "xq_f32": np.ascontiguousarray(xq),
                "a_aug": a_aug,
                "wv_dup": wv_dup,
                "bv8": bv8,
            }
        )
    return in_maps


_NC_CACHE = None


def kernel(**inputs) -> np.ndarray:
    global _NC_CACHE
    from concourse.bass_utils import run_bass_kernel_spmd

    if _NC_CACHE is None:
        _NC_CACHE = build_nc()
    nc = _NC_CACHE
    in_maps = host_prep(inputs)
    res = run_bass_kernel_spmd(nc, in_maps, core_ids=list(range(N_CORES)))
    x = np.asarray(inputs["x"], np.float32)
    full = np.empty((B, C, N), np.float32)
    for core in range(N_CORES):
        b, h = core // 2, core % 2
        full[b][:, h * NQ : (h + 1) * NQ] = res.results[core]["out"]
    return full.reshape(x.shape)


if __name__ == "__main__":
    rng = np.random.default_rng(0)
    demo = {
        "x": rng.standard_normal((B, C, 16, 16, 16), dtype=np.float32),
        "wq": 0.05 * rng.standard_normal((8, C), dtype=np.float32),
        "bq": 0.05 * rng.standard_normal((8,), dtype=np.float32),
        "wk": 0.05 * rng.standard_normal((8, C), dtype=np.float32),
        "bk": 0.05 * rng.standard_normal((8,), dtype=np.float32),
        "wv": 0.05 * rng.standard_normal((C, C), dtype=np.float32),
        "bv": 0.05 * rng.standard_normal((C,), dtype=np.float32),
        "gamma": np.zeros((1,), np.float32),
    }
    print(kernel(**demo).shape)
